# revision 1
# baseline (speedup 1.0000x reference)
"""Trainium2 Bass kernel for nn_ExplicitSVDBlock (dense transformer block).

Sharding: 8 NeuronCores = 4 batches x 2 query-halves of 1024 tokens.
Each core receives its batch's full 2048 tokens (permuted so its own
query tokens come first), redundantly builds K/V for all keys, and
computes everything else for its 1024 query tokens.  Zero cross-core
communication; host gathers the 8 [1024, 768] shards.

Device program: feature-major activations for matmuls (PE transposes
bridge to token-major for layernorm/residual), float32r matmul dtype,
softmax via exp on ScalarE with a [V | 1]-augmented stationary so the
denominators come out of the same PE accumulation.
"""
import sys

if '/opt/trn_rl_repo' not in sys.path:
    sys.path.insert(0, '/opt/trn_rl_repo')

import numpy as np
import concourse.bass as bass
import concourse.bacc as bacc
import concourse.mybir as mybir
import concourse.tile as tile
from concourse.bass_utils import run_bass_kernel_spmd
from concourse.masks import make_identity

F32 = mybir.dt.float32
F32R = mybir.dt.float32r
AF = mybir.ActivationFunctionType
OP = mybir.AluOpType

B, S, D, H, HD, RA = 4, 2048, 768, 12, 64, 32
RF, DFF = 512, 3072
P = 128
SK, SQ = S, S // 2          # keys per core / queries per core
HRA = H * RA                # 384
MT_D = D // P               # 6
KT_A = HRA // P             # 3
NKT = SK // P               # 16
NQT = SQ // P               # 8
QCH = 256                   # attention query chunk
NQC = SQ // QCH
KB = 4                      # score k-tiles per exp batch
MT_RF = RF // P             # 4
MT_DFF = DFF // P           # 24
NDCH = DFF // 512           # 6
TCH = 256                   # build token chunk
TCH3 = 512                  # post-attention token chunk
SKH = SK // 2
LN_EPS = 1e-6
N_CORES = 8

_CACHE = {}
import os
_PHASES = int(os.environ.get("BASS_KERNEL_PHASES", "4"))


def _declare_io(nc):
    t = {}

    def inp(name, shape):
        t[name] = nc.dram_tensor(name, list(shape), F32, kind="ExternalInput")

    inp("xfull", (SK, D))
    inp("cos2", (P, SK))
    inp("sin2", (P, SK))
    for p in ("q", "k", "v"):
        inp(f"ucat_{p}", (D, HRA))
    for p in ("q", "qr", "k", "kr"):
        inp(f"bdv_{p}", (MT_D, P, P))
        inp(f"bias_{p}", (P, MT_D))
    inp("bdvv", (HRA, D))
    inp("bv", (D,))
    inp("wot", (D, D))
    inp("wo_b", (D,))
    inp("ui", (D, RF))
    inp("vi", (RF, 2 * DFF))
    inp("bi1t", (P, MT_DFF))
    inp("bi2t", (P, MT_DFF))
    inp("uo", (DFF, RF))
    inp("vo", (RF, D))
    inp("bo", (D,))
    t["out"] = nc.dram_tensor("out", [SQ, D], F32, kind="ExternalOutput")
    t["nrm"] = nc.dram_tensor("nrm_scratch", [H, NQC, QCH], F32)  # internal
    return t


def _bcast_ap(dram_tensor, n):
    return bass.AP(dram_tensor.ap().tensor, 0, [[0, P], [1, n]])


def _emit(nc, tc, t):
    rsc = float(1.0 / np.sqrt(HD))

    const_cm = tc.tile_pool(name="const", bufs=1)
    const = const_cm.__enter__()
    ident = const.tile([P, P], F32)
    make_identity(nc, ident)

    poolQKV_cm = tc.tile_pool(name="pQKV", bufs=1)
    poolQKV = poolQKV_cm.__enter__()
    qTr = poolQKV.tile([P, MT_D, SQ], F32R)
    kTr = poolQKV.tile([P, MT_D, SK], F32R)
    vaug = poolQKV.tile([P, NKT, H * (HD + 1)], F32R)
    vaug4 = vaug[:].rearrange("p n (h e) -> p n h e", h=H)

    # ---- phase 1: LN1 + QKV build ----
    with tc.tile_pool(name="bw", bufs=1) as wpool, \
         tc.tile_pool(name="bh", bufs=2) as hpool, \
         tc.tile_pool(name="bxu", bufs=2) as xupool, \
         tc.tile_pool(name="brot", bufs=1) as rotpool, \
         tc.tile_pool(name="bx", bufs=2) as xpool, \
         tc.tile_pool(name="bst", bufs=3) as stpool, \
         tc.tile_pool(name="psA", bufs=3, space="PSUM") as psA, \
         tc.tile_pool(name="psB", bufs=2, space="PSUM") as psB, \
         tc.tile_pool(name="psV", bufs=1, space="PSUM") as psV:

        ucat, bdv, bias = {}, {}, {}
        for p in ("q", "k", "v"):
            w = wpool.tile([P, MT_D, HRA], F32R, tag=f"ucat_{p}")
            nc.sync.dma_start(w[:], t[f"ucat_{p}"].ap().rearrange(
                "(kt p) m -> p kt m", p=P).bitcast(F32R))
            ucat[p] = w
        for p in ("q", "qr", "k", "kr"):
            w = wpool.tile([P, MT_D, P], F32R, tag=f"bdv_{p}")
            nc.sync.dma_start(w[:], t[f"bdv_{p}"].ap().rearrange(
                "m p x -> p m x").bitcast(F32R))
            bdv[p] = w
            bl = wpool.tile([P, MT_D], F32, tag=f"bias_{p}")
            nc.sync.dma_start(bl[:], t[f"bias_{p}"].ap())
            bias[p] = bl
        bdvv = wpool.tile([P, KT_A, D], F32R)
        nc.sync.dma_start(bdvv[:], t["bdvv"].ap().rearrange(
            "(kt p) d -> p kt d", p=P).bitcast(F32R))
        bv_bc = wpool.tile([P, D], F32)
        nc.gpsimd.dma_start(bv_bc[:], _bcast_ap(t["bv"], D))
        eps_t = wpool.tile([P, 1], F32)
        nc.vector.memset(eps_t[:], LN_EPS)
        ones_h = wpool.tile([P, H], F32)
        nc.vector.memset(ones_h[:], 1.0)
        for _kt in range(NKT):
            nc.vector.tensor_copy(vaug4[:, _kt, :, HD], ones_h[:])

        for half in range(2):
            goff = half * SKH
            for tch in range(SKH // TCH):
                coff = tch * TCH
                gcoff = goff + coff
                hT = hpool.tile([P, MT_D, TCH], F32R, tag="hT")
                cosc = hpool.tile([P, TCH], F32, tag="cosc")
                sinc = hpool.tile([P, TCH], F32, tag="sinc")
                nc.sync.dma_start(cosc[:], t["cos2"][:, gcoff:gcoff + TCH])
                nc.sync.dma_start(sinc[:], t["sin2"][:, gcoff:gcoff + TCH])

                for tb in range(TCH // P):
                    x_t = xpool.tile([P, D], F32, tag="x_t")
                    r0 = gcoff + tb * P
                    nc.sync.dma_start(x_t[:], t["xfull"][r0:r0 + P, :])
                    xg = x_t[:].rearrange("p (n s) -> p n s", s=256)
                    stats = stpool.tile([P, D // 256, 6], F32, tag="stats")
                    for g in range(D // 256):
                        nc.vector.bn_stats(stats[:, g, :], xg[:, g, :])
                    mv = stpool.tile([P, 2], F32, tag="mv")
                    nc.vector.bn_aggr(mv[:], stats[:])
                    rstd = stpool.tile([P, 1], F32, tag="rstd")
                    nc.scalar.activation(rstd[:], mv[:, 1:2], AF.Sqrt, bias=eps_t[:])
                    nc.vector.reciprocal(rstd[:], rstd[:])
                    nc.vector.tensor_scalar(x_t[:], x_t[:], mv[:, 0:1], rstd[:],
                                            OP.subtract, OP.mult)
                    for mg in range(MT_D // 3):
                        ps = psA.tile([P, 3, P], F32, tag="b1")
                        for j in range(3):
                            mt = mg * 3 + j
                            nc.tensor.transpose(ps[:, j, :],
                                                x_t[:, mt * P:(mt + 1) * P], ident[:])
                        nc.scalar.activation(
                            hT[:, mg * 3:(mg + 1) * 3, tb * P:(tb + 1) * P],
                            ps[:], AF.Copy)

                projs = ["k", "v"] + (["q"] if half == 0 else [])
                for p in projs:
                    xs = xupool.tile([P, KT_A, TCH], F32R, tag="xu_sb")
                    for ma in range(KT_A):
                        xps = psA.tile([P, TCH], F32, tag="b1")
                        for kt in range(MT_D):
                            nc.tensor.matmul(xps[:], ucat[p][:, kt, ma * P:(ma + 1) * P],
                                             hT[:, kt, :],
                                             start=(kt == 0), stop=(kt == MT_D - 1))
                        nc.scalar.activation(xs[:, ma, :], xps[:], AF.Copy)
                    if p == "v":
                        for tb in range(TCH // P):
                            vps = psV.tile([P, D], F32, tag="v_ps")
                            for n0 in range(0, D, 512):
                                n1 = min(n0 + 512, D)
                                for ka in range(KT_A):
                                    nc.tensor.matmul(vps[:, n0:n1],
                                                     xs[:, ka, tb * P:(tb + 1) * P],
                                                     bdvv[:, ka, n0:n1],
                                                     start=(ka == 0),
                                                     stop=(ka == KT_A - 1))
                            ktg = gcoff // P + tb
                            nc.vector.tensor_tensor(
                                vaug4[:, ktg, :, 0:HD],
                                vps[:].rearrange("p (h e) -> p h e", h=H),
                                bv_bc[:].rearrange("p (h e) -> p h e", h=H),
                                OP.add)
                    else:
                        dst = qTr if p == "q" else kTr
                        dcols = slice(coff, coff + TCH) if p == "q" else \
                                slice(gcoff, gcoff + TCH)
                        rot = rotpool.tile([P, MT_D, TCH], F32R, tag="rot")
                        for m in range(MT_D):
                            ps2 = psB.tile([P, TCH], F32, tag="st2")
                            nc.tensor.matmul(ps2[:], bdv[p][:, m, :], xs[:, m // 2, :],
                                             start=True, stop=True)
                            nc.scalar.activation(dst[:, m, dcols], ps2[:], AF.Identity,
                                                 bias=bias[p][:, m:m + 1])
                            ps3 = psB.tile([P, TCH], F32, tag="st2")
                            nc.tensor.matmul(ps3[:], bdv[p + "r"][:, m, :],
                                             xs[:, m // 2, :], start=True, stop=True)
                            nc.vector.scalar_tensor_tensor(
                                rot[:, m, :], ps3[:], bias[p + "r"][:, m:m + 1],
                                sinc[:], OP.add, OP.mult)
                        dsl = dst[:, :, dcols]
                        cb = cosc[:, None, :].to_broadcast([P, MT_D, TCH])
                        nc.vector.tensor_tensor(dsl, dsl, cb, OP.mult)
                        nc.vector.tensor_tensor(dsl, dsl, rot[:], OP.add)

    # ---- phase 2: attention ----
    if _PHASES < 2:
        poolQKV_cm.__exit__(None, None, None)
        with tc.tile_pool(name="fb", bufs=2) as fb:
            for tt in range(NQT):
                ft = fb.tile([P, D], F32, tag="ft")
                nc.sync.dma_start(ft[:], t["xfull"][tt * P:(tt + 1) * P, :])
                nc.sync.dma_start(t["out"][tt * P:(tt + 1) * P, :], ft[:])
        const_cm.__exit__(None, None, None)
        return
    poolO_cm = tc.tile_pool(name="pO", bufs=1, side="right")
    poolO = poolO_cm.__enter__()
    oTn = poolO.tile([64, H, SQ], F32R)

    with tc.tile_pool(name="aexp", bufs=2, side="right") as apool, \
         tc.tile_pool(name="anrm", bufs=3, side="right") as npool, \
         tc.tile_pool(name="psS", bufs=3, space="PSUM") as psS, \
         tc.tile_pool(name="psO", bufs=2, space="PSUM") as psO:
        for h in range(H):
            pair, hh = h // 2, h % 2
            rs = slice(hh * 64, hh * 64 + 64)
            for qc in range(NQC):
                qcols = slice(qc * QCH, (qc + 1) * QCH)
                expS = apool.tile([P, NKT, QCH], F32R, tag="expS")
                for kb in range(NKT // KB):
                    sps = psS.tile([P, KB, QCH], F32, tag="s_ps")
                    for j in range(KB):
                        kt = kb * KB + j
                        nc.tensor.matmul(sps[:, j, :],
                                         kTr[rs, pair, kt * P:(kt + 1) * P],
                                         qTr[rs, pair, qcols],
                                         start=True, stop=True)
                    nc.scalar.activation(expS[:, kb * KB:(kb + 1) * KB, :],
                                         sps[:], AF.Exp, scale=rsc)
                po = psO.tile([P, QCH], F32, tag="o_ps")
                for kt in range(NKT):
                    nc.tensor.matmul(po[0:HD + 1, :], vaug4[:, kt, h, :],
                                     expS[:, kt, :],
                                     start=(kt == 0), stop=(kt == NKT - 1))
                srow = npool.tile([P, QCH], F32, tag="srow")
                nc.vector.reciprocal(srow[HD:HD + 1, :], po[HD:HD + 1, :])
                # broadcast recip row across the 64 o-lanes via DRAM roundtrip
                nc.sync.dma_start(t["nrm"][h, qc, :], srow[HD:HD + 1, :])
                rb = npool.tile([64, QCH], F32, tag="rb")
                nc.gpsimd.dma_start(
                    rb[:], bass.AP(t["nrm"].ap().tensor,
                                   (h * NQC + qc) * QCH, [[0, 64], [1, QCH]]))
                nc.vector.tensor_tensor(oTn[:, h, qcols], po[0:HD, :],
                                        rb[:], OP.mult)
    poolQKV_cm.__exit__(None, None, None)

    # ---- phase 3: Wo + residual + LN2 ----
    if _PHASES < 3:
        poolO_cm.__exit__(None, None, None)
        with tc.tile_pool(name="fb", bufs=2) as fb:
            for tt in range(NQT):
                ft = fb.tile([P, D], F32, tag="ft")
                nc.sync.dma_start(ft[:], t["xfull"][tt * P:(tt + 1) * P, :])
                nc.sync.dma_start(t["out"][tt * P:(tt + 1) * P, :], ft[:])
        const_cm.__exit__(None, None, None)
        return
    poolX_cm = tc.tile_pool(name="pX", bufs=1)
    poolX = poolX_cm.__enter__()
    x1 = poolX.tile([P, NQT, D], F32)
    h2T = poolX.tile([P, MT_D, SQ], F32R)

    with tc.tile_pool(name="w3", bufs=1) as wp3, \
         tc.tile_pool(name="c3", bufs=2) as cp3, \
         tc.tile_pool(name="s3", bufs=4) as sp3, \
         tc.tile_pool(name="ps3", bufs=2, space="PSUM") as ps3, \
         tc.tile_pool(name="ps3b", bufs=3, space="PSUM") as ps3b:
        wot = wp3.tile([64, H, D], F32R)
        nc.sync.dma_start(wot[:], t["wot"].ap().rearrange(
            "(h p) d -> p h d", p=64).bitcast(F32R))
        wob_bc = wp3.tile([P, D], F32)
        nc.gpsimd.dma_start(wob_bc[:], _bcast_ap(t["wo_b"], D))
        bo_bc = wp3.tile([P, D], F32)
        nc.gpsimd.dma_start(bo_bc[:], _bcast_ap(t["bo"], D))
        eps3 = wp3.tile([P, 1], F32)
        nc.vector.memset(eps3[:], LN_EPS)

        for tch in range(SQ // TCH3):
            attT = cp3.tile([P, MT_D, TCH3], F32, tag="attT")
            for mt in range(MT_D):
                for n0 in range(0, TCH3, 512):
                    n1 = min(n0 + 512, TCH3)
                    aps = ps3.tile([P, 512], F32, tag="p31")
                    for h in range(H):
                        nc.tensor.matmul(aps[:, 0:n1 - n0],
                                         wot[:, h, mt * P:(mt + 1) * P],
                                         oTn[:, h, tch * TCH3 + n0:tch * TCH3 + n1],
                                         start=(h == 0), stop=(h == H - 1))
                    nc.scalar.activation(attT[:, mt, n0:n1], aps[:, 0:n1 - n0], AF.Copy)
            for tb in range(TCH3 // P):
                tt = (tch * TCH3) // P + tb
                tps3 = ps3b.tile([P, D], F32, tag="t3_ps")
                for mt in range(MT_D):
                    nc.tensor.transpose(tps3[:, mt * P:(mt + 1) * P],
                                        attT[:, mt, tb * P:(tb + 1) * P], ident[:])
                xq_t = sp3.tile([P, D], F32, tag="xq_t")
                nc.sync.dma_start(xq_t[:], t["xfull"][tt * P:(tt + 1) * P, :])
                nc.vector.tensor_tensor(xq_t[:], xq_t[:], wob_bc[:], OP.add)
                nc.vector.tensor_tensor(x1[:, tt, :], tps3[:], xq_t[:], OP.add)
                xg = x1[:, tt, :].rearrange("p (n s) -> p n s", s=256)
                stats = sp3.tile([P, D // 256, 6], F32, tag="st3")
                for g in range(D // 256):
                    nc.vector.bn_stats(stats[:, g, :], xg[:, g, :])
                mv = sp3.tile([P, 2], F32, tag="mv3")
                nc.vector.bn_aggr(mv[:], stats[:])
                rstd = sp3.tile([P, 1], F32, tag="rstd3")
                nc.scalar.activation(rstd[:], mv[:, 1:2], AF.Sqrt, bias=eps3[:])
                nc.vector.reciprocal(rstd[:], rstd[:])
                h2_t = sp3.tile([P, D], F32, tag="h2_t")
                nc.vector.tensor_scalar(h2_t[:], x1[:, tt, :], mv[:, 0:1], rstd[:],
                                        OP.subtract, OP.mult)
                nc.vector.tensor_tensor(x1[:, tt, :], x1[:, tt, :], bo_bc[:], OP.add)
                for mg in range(MT_D // 3):
                    ps = ps3.tile([P, 3, P], F32, tag="p31")
                    for j in range(3):
                        mt = mg * 3 + j
                        nc.tensor.transpose(ps[:, j, :], h2_t[:, mt * P:(mt + 1) * P],
                                            ident[:])
                    nc.scalar.activation(
                        h2T[:, mg * 3:(mg + 1) * 3, tt * P:(tt + 1) * P],
                        ps[:], AF.Copy)
    poolO_cm.__exit__(None, None, None)

    # ---- phase 4: FFN ----
    if _PHASES < 4:
        with tc.tile_pool(name="fb", bufs=2) as fb:
            for tt in range(NQT):
                ft = fb.tile([P, D], F32, tag="ft")
                nc.vector.tensor_copy(ft[:], x1[:, tt, :])
                nc.sync.dma_start(t["out"][tt * P:(tt + 1) * P, :], ft[:])
        poolX_cm.__exit__(None, None, None)
        const_cm.__exit__(None, None, None)
        return
    with tc.tile_pool(name="fw", bufs=1) as fw, \
         tc.tile_pool(name="fs", bufs=2) as fs, \
         tc.tile_pool(name="fcvi", bufs=2) as fcv, \
         tc.tile_pool(name="fc", bufs=2) as fc, \
         tc.tile_pool(name="psU", bufs=2, space="PSUM") as psU, \
         tc.tile_pool(name="psT", bufs=1, space="PSUM") as psT, \
         tc.tile_pool(name="psY", bufs=1, space="PSUM") as psY:
        ui = fw.tile([P, MT_D, RF], F32R)
        nc.sync.dma_start(ui[:], t["ui"].ap().rearrange(
            "(k p) m -> p k m", p=P).bitcast(F32R))
        vo = fw.tile([P, MT_RF, D], F32R)
        nc.sync.dma_start(vo[:], t["vo"].ap().rearrange(
            "(k p) m -> p k m", p=P).bitcast(F32R))
        bi1 = fw.tile([P, MT_DFF], F32)
        nc.sync.dma_start(bi1[:], t["bi1t"].ap())
        bi2 = fw.tile([P, MT_DFF], F32)
        nc.sync.dma_start(bi2[:], t["bi2t"].ap())

        for tch in range(SQ // TCH3):
            NT = TCH3
            w1T = fc.tile([P, MT_RF, NT], F32R, tag="w1T")
            for mt in range(MT_RF):
                for n0 in range(0, NT, 512):
                    n1 = min(n0 + 512, NT)
                    wps = psU.tile([P, 512], F32, tag="ups")
                    for kt in range(MT_D):
                        nc.tensor.matmul(wps[:, 0:n1 - n0],
                                         ui[:, kt, mt * P:(mt + 1) * P],
                                         h2T[:, kt, tch * NT + n0:tch * NT + n1],
                                         start=(kt == 0), stop=(kt == MT_D - 1))
                    nc.scalar.activation(w1T[:, mt, n0:n1], wps[:, 0:n1 - n0], AF.Copy)
            tps = psT.tile([P, MT_RF, 512], F32, tag="t_ps")
            for dch in range(NDCH):
                vi1 = fcv.tile([P, 4, 512], F32R, tag="vi1")
                nc.sync.dma_start(vi1[:], t["vi"].ap()[:, dch * 512:(dch + 1) * 512]
                                  .rearrange("(k p) m -> p k m", p=P).bitcast(F32R))
                vi2 = fcv.tile([P, 4, 512], F32R, tag="vi2")
                nc.sync.dma_start(vi2[:],
                                  t["vi"].ap()[:, DFF + dch * 512:DFF + (dch + 1) * 512]
                                  .rearrange("(k p) m -> p k m", p=P).bitcast(F32R))
                uoc = fcv.tile([P, 4, RF], F32R, tag="uoc")
                nc.sync.dma_start(uoc[:], t["uo"].ap()[dch * 512:(dch + 1) * 512, :]
                                  .rearrange("(k p) m -> p k m", p=P).bitcast(F32R))
                g = fs.tile([P, 4, NT], F32R, tag="g")
                for m4 in range(4):
                    bcol = dch * 4 + m4
                    for n0 in range(0, NT, 512):
                        n1 = min(n0 + 512, NT)
                        u1ps = psU.tile([P, 512], F32, tag="ups")
                        for kt in range(MT_RF):
                            nc.tensor.matmul(u1ps[:, 0:n1 - n0],
                                             vi1[:, kt, m4 * P:(m4 + 1) * P],
                                             w1T[:, kt, n0:n1],
                                             start=(kt == 0), stop=(kt == MT_RF - 1))
                        nc.scalar.activation(g[:, m4, n0:n1], u1ps[:, 0:n1 - n0],
                                             AF.Gelu_apprx_tanh,
                                             bias=bi1[:, bcol:bcol + 1])
                        u2ps = psU.tile([P, 512], F32, tag="ups")
                        for kt in range(MT_RF):
                            nc.tensor.matmul(u2ps[:, 0:n1 - n0],
                                             vi2[:, kt, m4 * P:(m4 + 1) * P],
                                             w1T[:, kt, n0:n1],
                                             start=(kt == 0), stop=(kt == MT_RF - 1))
                        nc.vector.scalar_tensor_tensor(g[:, m4, n0:n1],
                                                       u2ps[:, 0:n1 - n0],
                                                       bi2[:, bcol:bcol + 1],
                                                       g[:, m4, n0:n1],
                                                       OP.add, OP.mult)
                for mr in range(MT_RF):
                    for ktl in range(4):
                        nc.tensor.matmul(tps[:, mr, 0:NT],
                                         uoc[:, ktl, mr * P:(mr + 1) * P],
                                         g[:, ktl, :],
                                         start=(dch == 0 and ktl == 0),
                                         stop=(dch == NDCH - 1 and ktl == 3),
                                         skip_group_check=True)
            tT = fc.tile([P, MT_RF, NT], F32R, tag="tT")
            nc.scalar.activation(tT[:], tps[:, :, 0:NT], AF.Copy)
            yT = fc.tile([P, MT_D, NT], F32, tag="yT")
            for mt in range(MT_D):
                for n0 in range(0, NT, 512):
                    n1 = min(n0 + 512, NT)
                    yps = psU.tile([P, 512], F32, tag="ups")
                    for kt in range(MT_RF):
                        nc.tensor.matmul(yps[:, 0:n1 - n0],
                                         vo[:, kt, mt * P:(mt + 1) * P],
                                         tT[:, kt, n0:n1],
                                         start=(kt == 0), stop=(kt == MT_RF - 1))
                    nc.scalar.activation(yT[:, mt, n0:n1], yps[:, 0:n1 - n0], AF.Copy)
            for tb in range(NT // P):
                tt = (tch * NT) // P + tb
                yps2 = psY.tile([P, D], F32, tag="yt_ps")
                for mt in range(MT_D):
                    nc.tensor.transpose(yps2[:, mt * P:(mt + 1) * P],
                                        yT[:, mt, tb * P:(tb + 1) * P], ident[:])
                o_t = fc.tile([P, D], F32, tag="o_t")
                nc.vector.tensor_tensor(o_t[:], yps2[:], x1[:, tt, :], OP.add)
                nc.sync.dma_start(t["out"][tt * P:(tt + 1) * P, :], o_t[:])
    poolX_cm.__exit__(None, None, None)
    const_cm.__exit__(None, None, None)


def _build_module():
    nc = bacc.Bacc("TRN2", target_bir_lowering=False, debug=False, num_devices=N_CORES)
    t = _declare_io(nc)
    with tile.TileContext(nc) as tc:
        _emit(nc, tc, t)
    nc.compile()
    return nc


def _prep_weights(inputs):
    def rot_last(a):
        return np.concatenate([-a[..., HD // 2:], a[..., :HD // 2]], axis=-1)

    f32 = lambda a: np.ascontiguousarray(np.asarray(a), dtype=np.float32)
    w = {}
    for p, U, V, b in (("q", inputs["Uq"], inputs["Vq"], inputs["bq"]),
                       ("k", inputs["Uk"], inputs["Vk"], inputs["bk"])):
        U, V, b = f32(U), f32(V), f32(b)
        w[f"ucat_{p}"] = f32(U.transpose(1, 0, 2).reshape(D, HRA))
        for suf, VV in ((p, V), (p + "r", rot_last(V))):
            blk = np.zeros((MT_D, P, P), np.float32)
            for m in range(MT_D):
                for j in range(2):
                    h = 2 * m + j
                    ro = (h % 4) * RA
                    blk[m, ro:ro + RA, 64 * j:64 * j + HD] = VV[h]
            w[f"bdv_{suf}"] = blk
        w[f"bias_{p}"] = f32(b.reshape(MT_D, P).T)
        w[f"bias_{p}r"] = f32(rot_last(b.reshape(H, HD)).reshape(D).reshape(MT_D, P).T)
    w["ucat_v"] = f32(f32(inputs["Uv"]).transpose(1, 0, 2).reshape(D, HRA))
    bdvv = np.zeros((HRA, D), np.float32)
    Vv = f32(inputs["Vv"])
    for h in range(H):
        bdvv[h * RA:(h + 1) * RA, h * HD:(h + 1) * HD] = Vv[h]
    w["bdvv"] = bdvv
    w["bv"] = f32(inputs["bv"])
    w["wot"] = f32(f32(inputs["Wo_w"]).T)
    w["wo_b"] = f32(inputs["Wo_b"])
    w["ui"] = f32(inputs["Ui"])
    w["vi"] = f32(inputs["Vi"])
    bi = f32(inputs["bi"])
    w["bi1t"] = f32(bi[:DFF].reshape(MT_DFF, P).T)
    w["bi2t"] = f32(bi[DFF:].reshape(MT_DFF, P).T)
    w["uo"] = f32(inputs["Uo"])
    w["vo"] = f32(inputs["Vo"])
    w["bo"] = f32(inputs["bo"])
    return w


def _make_inmaps(inputs):
    w = _prep_weights(inputs)
    x = np.asarray(inputs["x"], dtype=np.float32)
    cos = np.asarray(inputs["cos"], dtype=np.float32)
    sin = np.asarray(inputs["sin"], dtype=np.float32)
    in_maps = []
    for core in range(N_CORES):
        b, hf = core // 2, core % 2
        sel = np.r_[hf * SQ:(hf + 1) * SQ, (1 - hf) * SQ:(2 - hf) * SQ]
        m = dict(w)
        m["xfull"] = np.ascontiguousarray(x[b][sel])
        cp, sp = cos[sel].T, sin[sel].T
        m["cos2"] = np.ascontiguousarray(np.concatenate([cp, cp], 0))
        m["sin2"] = np.ascontiguousarray(np.concatenate([sp, sp], 0))
        in_maps.append(m)
    return in_maps


def _run(inputs, **kwargs):
    nc = _CACHE.get("nc")
    if nc is None:
        nc = _CACHE["nc"] = _build_module()
    in_maps = _make_inmaps(inputs)
    res = run_bass_kernel_spmd(nc, in_maps, list(range(N_CORES)), **kwargs)
    out = np.empty((B, S, D), np.float32)
    for core in range(N_CORES):
        b, hf = core // 2, core % 2
        out[b, hf * SQ:(hf + 1) * SQ] = res.results[core]["out"]
    return out, res


def kernel(**inputs):
    out, _ = _run(inputs)
    return out



# revision 47
# speedup vs baseline: 1.6670x; 1.6670x over previous
"""Trainium2 Bass kernel for nn_ExplicitSVDBlock (dense transformer block).

Sharding: 8 NeuronCores = 4 batches x 2 query-halves of 1024 tokens.
Each core receives its batch's full 2048 tokens (permuted so its own
query tokens come first), redundantly builds K/V for all keys, and
computes everything else for its 1024 query tokens.  Zero cross-core
communication; host gathers the 8 [1024, 768] shards.

Device program (v2): bf16 activations/weights with fp32 PSUM accumulation,
DMA-xbar transposes for all token<->feature layout flips, fused
attention -> Wo -> LN2 pipeline per 256-token query chunk with pair-packed
token-major Wo (bias folded into a ones-row matmul), softmax denominators
broadcast via a PE rank-1 matmul, FFN weights chunk-streamed on the Pool
SWDGE queue.
"""
import sys

if '/opt/trn_rl_repo' not in sys.path:
    sys.path.insert(0, '/opt/trn_rl_repo')

import numpy as np
import ml_dtypes
import concourse.bass as bass
import concourse.bacc as bacc
import concourse.mybir as mybir
import concourse.tile as tile
from concourse.bass_utils import run_bass_kernel_spmd

F32 = mybir.dt.float32
BF = mybir.dt.bfloat16
F16 = mybir.dt.float16
AF = mybir.ActivationFunctionType
OP = mybir.AluOpType

B, S, D, H, HD, RA = 4, 2048, 768, 12, 64, 32
RF, DFF = 512, 3072
P = 128
SK, SQ = S, S // 2          # keys per core / queries per core
HRA = H * RA                # 384
MT_D = D // P               # 6
KT_A = HRA // P             # 3
NKT = SK // P               # 16
NQT = SQ // P               # 8
QCH = 256                   # attention query chunk
NQC = SQ // QCH             # 4
NPAIR = H // 2              # 6
MT_RF = RF // P             # 4
MT_DFF = DFF // P           # 24
NDCH = DFF // 512           # 6
NT = 256                    # FFN token chunk (one attention query chunk)
TCH = 512                   # phase-1 token chunk
SKH = SK // 2
LN_EPS = 1e-6
N_CORES = 8

_CACHE = {}


def _declare_io(nc):
    t = {}

    def inp(name, shape, dt=F16):
        t[name] = nc.dram_tensor(name, list(shape), dt, kind="ExternalInput")

    inp("xfull", (SK, D))
    inp("cos2", (P, SK))
    inp("sin2", (P, SK))
    for p in ("q", "k", "v"):
        inp(f"ucat_{p}", (P, MT_D, HRA))
    for p in ("q", "qr", "k", "kr"):
        inp(f"bdv_{p}", (P, MT_D, P))
        inp(f"bias_{p}", (P, MT_D), F32)
    inp("bdvv", (P, KT_A, D))
    inp("bv", (D,), F32)
    inp("wotP", (P, NPAIR, D))
    inp("wob_row", (1, D))
    inp("bo_row", (1, D))
    inp("ui", (P, MT_D, RF))
    inp("vi", (P, MT_RF, 2 * DFF))
    inp("uo", (P, NDCH, MT_RF, RF))
    inp("vo", (P, MT_RF, D))
    inp("bi1t", (P, MT_DFF), F32)
    inp("bi2t", (P, MT_DFF), F32)
    t["out"] = nc.dram_tensor("out", [SQ, D], F32, kind="ExternalOutput")
    return t


def _bcast_ap(dram_tensor, n):
    return bass.AP(dram_tensor.ap().tensor, 0, [[0, P], [1, n]])


def _emit(nc, tc, t):
    rsc = float(1.0 / np.sqrt(HD))

    poolR_cm = tc.tile_pool(name="res", bufs=1)
    poolR = poolR_cm.__enter__()
    qTr = poolR.tile([P, MT_D, SQ], F16)
    kTr = poolR.tile([P, MT_D, SK], F16)
    vaug = poolR.tile([P, NKT, H * (HD + 1)], BF)
    vaug4 = vaug[:].rearrange("p n (h e) -> p n h e", h=H)
    oT = poolR.tile([P, NPAIR, SQ], F16)
    x1 = poolR.tile([P, NQT, D], F16)
    h2T = poolR.tile([P, MT_D, SQ], F16)
    wotP = poolR.tile([P, NPAIR, D], F16)
    wob_r = poolR.tile([1, D], F16)
    bo_r = poolR.tile([1, D], F16)
    ones1 = poolR.tile([1, P], F16)
    ones_b = poolR.tile([1, HD], BF)
    ui_t = poolR.tile([P, MT_D, RF], F16)
    vo_t = poolR.tile([P, MT_RF, D], F16)
    bi1 = poolR.tile([P, MT_DFF], F32)
    bi2 = poolR.tile([P, MT_DFF], F32)
    eps_t = poolR.tile([P, 1], F32)

    nc.vector.memset(ones1[:], 1.0)
    nc.vector.memset(ones_b[:], 1.0)
    nc.vector.memset(eps_t[:], LN_EPS)

    # ---- phase 1: LN1 + QKV build ----
    with tc.tile_pool(name="bw", bufs=1) as wpool, \
         tc.tile_pool(name="bh", bufs=3) as hpool, \
         tc.tile_pool(name="bxu", bufs=2) as xupool, \
         tc.tile_pool(name="brot", bufs=2) as rotpool, \
         tc.tile_pool(name="bx", bufs=5) as xpool, \
         tc.tile_pool(name="bst", bufs=3) as stpool, \
         tc.tile_pool(name="psA", bufs=2, space="PSUM") as psA, \
         tc.tile_pool(name="psB", bufs=2, space="PSUM") as psB, \
         tc.tile_pool(name="psV", bufs=2, space="PSUM") as psV:

        # phase-1 weights on the Pool SWDGE queue (no HWDGE contention, Act
        # and SP stay free for the LN chain); k first: first projection used
        ucat, bdv, bias = {}, {}, {}
        for p in ("k", "v", "q"):
            w = wpool.tile([P, MT_D, HRA], F16, tag=f"ucat_{p}")
            ucat[p] = w
        for p in ("k", "kr", "q", "qr"):
            w = wpool.tile([P, MT_D, P], F16, tag=f"bdv_{p}")
            bdv[p] = w
            bl = wpool.tile([P, MT_D], F32, tag=f"bias_{p}")
            bias[p] = bl
        bdvv = wpool.tile([P, KT_A, D], F16)
        bv_bc = wpool.tile([P, D], F32)
        for p in ("k", "kr"):
            nc.gpsimd.dma_start(bdv[p][:], t[f"bdv_{p}"].ap())
            nc.gpsimd.dma_start(bias[p][:], t[f"bias_{p}"].ap())
        nc.gpsimd.dma_start(ucat["k"][:], t["ucat_k"].ap())
        nc.gpsimd.dma_start(ucat["v"][:], t["ucat_v"].ap())
        nc.gpsimd.dma_start(bdvv[:], t["bdvv"].ap())
        nc.gpsimd.dma_start(ucat["q"][:], t["ucat_q"].ap())
        for p in ("q", "qr"):
            nc.gpsimd.dma_start(bdv[p][:], t[f"bdv_{p}"].ap())
            nc.gpsimd.dma_start(bias[p][:], t[f"bias_{p}"].ap())
        nc.gpsimd.dma_start(bv_bc[:], _bcast_ap(t["bv"], D))
        ones_h = wpool.tile([P, H], BF)
        nc.vector.memset(ones_h[:], 1.0)
        for _kt in range(NKT):
            nc.vector.tensor_copy(vaug4[:, _kt, :, HD], ones_h[:])

        # software-prefetched x tiles (sync queue)
        xtiles = {}

        def load_x(gidx):
            xb = xpool.tile([P, D], F16, tag="xb")
            nc.sync.dma_start(xb[:], t["xfull"][gidx * P:(gidx + 1) * P, :])
            xtiles[gidx] = xb

        for _g in range(4):
            load_x(_g)

        chunks = [(h, c) for h in range(2) for c in range(SKH // TCH)]
        hts = {}

        def emit_prefetch():
            # resident weights for the later phases: issued after chunk 0 so
            # their transfers stay off the startup critical path
            nc.gpsimd.dma_start(wotP[:], t["wotP"].ap())
            nc.gpsimd.dma_start(wob_r[:], t["wob_row"].ap())
            nc.gpsimd.dma_start(bo_r[:], t["bo_row"].ap())
            nc.gpsimd.dma_start(ui_t[:], t["ui"].ap())
            nc.gpsimd.dma_start(vo_t[:], t["vo"].ap())
            nc.gpsimd.dma_start(bi1[:], t["bi1t"].ap())
            nc.gpsimd.dma_start(bi2[:], t["bi2t"].ap())

        def ln_chunk(ci):
            half, tch = chunks[ci]
            gcoff = half * SKH + tch * TCH
            hT = hpool.tile([P, MT_D, TCH], F16, tag="hT")
            cosc = hpool.tile([P, TCH], F16, tag="cosc")
            sinc = hpool.tile([P, TCH], F16, tag="sinc")
            nc.sync.dma_start(cosc[:], t["cos2"][:, gcoff:gcoff + TCH])
            nc.sync.dma_start(sinc[:], t["sin2"][:, gcoff:gcoff + TCH])
            hts[ci] = (hT, cosc, sinc)
            for tb in range(TCH // P):
                g0 = gcoff // P + tb
                x_t = xtiles.pop(g0)
                xg = x_t[:].rearrange("p (n s) -> p n s", s=256)
                stats = stpool.tile([P, D // 256, 6], F32, tag="stats")
                for g in range(D // 256):
                    nc.vector.bn_stats(stats[:, g, :], xg[:, g, :])
                mv = stpool.tile([P, 2], F32, tag="mv")
                nc.vector.bn_aggr(mv[:], stats[:])
                sig = stpool.tile([P, 1], F32, tag="sig")
                nc.scalar.activation(sig[:], mv[:, 1:2], AF.Sqrt, bias=eps_t[:])
                nc.vector.reciprocal(sig[:], sig[:])
                xhat = xpool.tile([P, D], F16, tag="xhat")
                nc.vector.tensor_scalar(xhat[:], x_t[:], mv[:, 0:1], sig[:],
                                        OP.subtract, OP.mult)
                nc.sync.dma_start_transpose(hT[:, :, tb * P:(tb + 1) * P],
                                            xhat[:])
                if g0 + 4 < NKT:
                    load_x(g0 + 4)

        def proj_chunk(ci):
            half, tch = chunks[ci]
            coff = tch * TCH
            gcoff = half * SKH + coff
            hT, cosc, sinc = hts.pop(ci)
            projs = ["k", "v"] + (["q"] if half == 0 else [])
            for p in projs:
                    xs = xupool.tile([P, KT_A, TCH], F16, tag="xu_sb")
                    for ma in range(KT_A):
                        xps = psA.tile([P, TCH], F32, tag="b1")
                        for kt in range(MT_D):
                            nc.tensor.matmul(xps[:], ucat[p][:, kt, ma * P:(ma + 1) * P],
                                             hT[:, kt, :],
                                             start=(kt == 0), stop=(kt == MT_D - 1))
                        nc.scalar.activation(xs[:, ma, :], xps[:], AF.Copy)
                    if p == "v":
                        for tb in range(TCH // P):
                            vps = psV.tile([P, D], F32, tag="v_ps")
                            for n0 in range(0, D, 512):
                                n1 = min(n0 + 512, D)
                                for ka in range(KT_A):
                                    nc.tensor.matmul(vps[:, n0:n1],
                                                     xs[:, ka, tb * P:(tb + 1) * P],
                                                     bdvv[:, ka, n0:n1],
                                                     start=(ka == 0),
                                                     stop=(ka == KT_A - 1))
                            ktg = gcoff // P + tb
                            nc.vector.tensor_tensor(
                                vaug4[:, ktg, :, 0:HD],
                                vps[:].rearrange("p (h e) -> p h e", h=H),
                                bv_bc[:].rearrange("p (h e) -> p h e", h=H),
                                OP.add)
                    else:
                        dst = qTr if p == "q" else kTr
                        dcols = slice(coff, coff + TCH) if p == "q" else \
                                slice(gcoff, gcoff + TCH)
                        rot = rotpool.tile([P, MT_D, TCH], F16, tag="rot")
                        for m in range(MT_D):
                            ps2 = psB.tile([P, TCH], F32, tag="st2")
                            nc.tensor.matmul(ps2[:], bdv[p][:, m, :], xs[:, m // 2, :],
                                             start=True, stop=True)
                            nc.scalar.activation(dst[:, m, dcols], ps2[:], AF.Identity,
                                                 bias=bias[p][:, m:m + 1])
                            ps3 = psB.tile([P, TCH], F32, tag="st2")
                            nc.tensor.matmul(ps3[:], bdv[p + "r"][:, m, :],
                                             xs[:, m // 2, :], start=True, stop=True)
                            nc.vector.scalar_tensor_tensor(
                                rot[:, m, :], ps3[:], bias[p + "r"][:, m:m + 1],
                                sinc[:], OP.add, OP.mult)
                        dsl = dst[:, :, dcols]
                        cb = cosc[:, None, :].to_broadcast([P, MT_D, TCH])
                        nc.vector.tensor_tensor(dsl, dsl, cb, OP.mult)
                        nc.vector.tensor_tensor(dsl, dsl, rot[:], OP.add)

        # pipelined emission: LN(c+1) queued ahead of projections(c) so the
        # in-order Act/DVE queues never block the next chunk's LN chain
        for ci in range(len(chunks)):
            ln_chunk(ci)
            if ci == 0:
                emit_prefetch()
            if ci >= 1:
                proj_chunk(ci - 1)
        proj_chunk(len(chunks) - 1)

    # ---- attention phase: scores/exp/AV + Wo + residual (pure-Exp on Act) ----
    mv8 = poolR.tile([P, NQT, 2], F32)
    with tc.tile_pool(name="aexp", bufs=2, side="right") as apool, \
         tc.tile_pool(name="anrm", bufs=3, side="right") as npool, \
         tc.tile_pool(name="psS", bufs=2, space="PSUM") as psS, \
         tc.tile_pool(name="psO", bufs=2, space="PSUM") as psO, \
         tc.tile_pool(name="pw", bufs=2, space="PSUM") as pw:

        def emit_scores(h, qc, expS):
            pair, hh = divmod(h, 2)
            rs = slice(hh * 64, hh * 64 + 64)
            qcols = slice(qc * QCH, (qc + 1) * QCH)
            for kb in range(NKT // 4):
                sps = psS.tile([P, 4, QCH], F32, tag="s_ps")
                for j in range(4):
                    kt = kb * 4 + j
                    nc.tensor.matmul(sps[:, j, :],
                                     kTr[rs, pair, kt * P:(kt + 1) * P],
                                     qTr[rs, pair, qcols],
                                     start=True, stop=True)
                nc.scalar.activation(expS[:, kb * 4:(kb + 1) * 4, :],
                                     sps[:], AF.Exp, scale=rsc)

        def emit_av(h, qc, expS):
            pair, hh = divmod(h, 2)
            qcols = slice(qc * QCH, (qc + 1) * QCH)
            po = psO.tile([P, 2, QCH], F32, tag="o_ps")
            for kt in range(NKT):
                nc.tensor.matmul(po[0:HD + 1, 0, :], vaug4[:, kt, h, :],
                                 expS[:, kt, :],
                                 start=(kt == 0), stop=(kt == NKT - 1))
            srow = npool.tile([1, QCH], BF, tag="srow")
            with nc.allow_low_precision(reason="softmax denom recip in bf16"):
                nc.vector.reciprocal(srow[:], po[HD:HD + 1, 0, :])
            nc.tensor.matmul(po[0:HD, 1, :], ones_b[0:1, :], srow[0:1, :],
                             start=True, stop=True)
            rbs = npool.tile([HD, QCH], BF, tag="rbs")
            nc.vector.tensor_copy(rbs[:], po[0:HD, 1, :])
            nc.vector.tensor_tensor(oT[hh * 64:hh * 64 + 64, pair, qcols],
                                    po[0:HD, 0, :], rbs[:], OP.mult)

        def emit_attention(qc):
            prev = None
            for h in range(H):
                expS = apool.tile([P, NKT, QCH], BF, tag="expS")
                emit_scores(h, qc, expS)
                if prev is not None:
                    emit_av(h - 1, qc, prev)
                prev = expS
            emit_av(H - 1, qc, prev)

        def emit_wo(tc_):
            # Wo + residual into x1 (no LN here: keeps this phase pure-Exp)
            for tb in range(QCH // P):
                tt = tc_ * (QCH // P) + tb
                tok = slice(tc_ * QCH + tb * P, tc_ * QCH + (tb + 1) * P)
                xb2 = npool.tile([P, D], F16, tag="xq")
                nc.gpsimd.dma_start(xb2[:], t["xfull"][tt * P:(tt + 1) * P, :])
                for c0 in range(0, D, 512):
                    c1 = min(c0 + 512, D)
                    aps = pw.tile([P, 512], F32, tag="w")
                    for pr in range(NPAIR):
                        nc.tensor.matmul(aps[:, 0:c1 - c0], oT[:, pr, tok],
                                         wotP[:, pr, c0:c1],
                                         start=(pr == 0), stop=False)
                    nc.tensor.matmul(aps[:, 0:c1 - c0], ones1[0:1, :],
                                     wob_r[0:1, c0:c1], start=False, stop=True)
                    nc.vector.tensor_tensor(x1[:, tt, c0:c1], aps[:, 0:c1 - c0],
                                            xb2[:, c0:c1], OP.add)
                # LN2 stats on DVE only (sqrt batched later in the FFN phase)
                xg = x1[:, tt, :].rearrange("p (n s) -> p n s", s=256)
                stats = npool.tile([P, D // 256, 6], F32, tag="st3")
                for g in range(D // 256):
                    nc.vector.bn_stats(stats[:, g, :], xg[:, g, :])
                nc.vector.bn_aggr(mv8[:, tt, :], stats[:])

        for qc in range(NQC):
            emit_attention(qc)
            emit_wo(qc)

    # ---- FFN phase: LN2 prelude then low-rank GEGLU ----
    NTF = 512
    with tc.tile_pool(name="fn", bufs=3, side="right") as npool, \
         tc.tile_pool(name="fs", bufs=2, side="right") as fs, \
         tc.tile_pool(name="fcvi", bufs=2) as fcv, \
         tc.tile_pool(name="psU", bufs=2, space="PSUM") as psU, \
         tc.tile_pool(name="psT", bufs=1, space="PSUM") as psT, \
         tc.tile_pool(name="psY", bufs=2, space="PSUM") as psY:
        # LN2 normalize: one batched Sqrt (single table load), then DVE + DMA
        sig8 = npool.tile([P, NQT], F32, tag="sig8")
        nc.scalar.activation(sig8[:], mv8[:, :, 1], AF.Sqrt, bias=eps_t[:])
        nc.vector.reciprocal(sig8[:], sig8[:])
        for tt in range(NQT):
            h2b = npool.tile([P, D], F16, tag="h2b")
            nc.vector.tensor_scalar(h2b[:], x1[:, tt, :], mv8[:, tt, 0:1],
                                    sig8[:, tt:tt + 1], OP.subtract, OP.mult)
            nc.sync.dma_start_transpose(h2T[:, :, tt * P:(tt + 1) * P], h2b[:])

        for tch in range(SQ // NTF):
            tcols = slice(tch * NTF, (tch + 1) * NTF)
            w1T = fs.tile([P, MT_RF, NTF], F16, tag="w1T")
            for mt in range(MT_RF):
                wps = psU.tile([P, NTF], F32, tag="ups")
                for kt in range(MT_D):
                    nc.tensor.matmul(wps[:], ui_t[:, kt, mt * P:(mt + 1) * P],
                                     h2T[:, kt, tcols],
                                     start=(kt == 0), stop=(kt == MT_D - 1))
                nc.scalar.activation(w1T[:, mt, :], wps[:], AF.Copy)
            tps = psT.tile([P, MT_RF, NTF], F32, tag="t_ps")
            for dch in range(NDCH):
                vi1 = fcv.tile([P, MT_RF, 512], F16, tag="vi1")
                nc.gpsimd.dma_start(vi1[:],
                                    t["vi"].ap()[:, :, dch * 512:(dch + 1) * 512])
                vi2 = fcv.tile([P, MT_RF, 512], F16, tag="vi2")
                nc.gpsimd.dma_start(
                    vi2[:], t["vi"].ap()[:, :, DFF + dch * 512:DFF + (dch + 1) * 512])
                uoc = fcv.tile([P, MT_RF, RF], F16, tag="uoc")
                nc.gpsimd.dma_start(uoc[:], t["uo"].ap()[:, dch, :, :])
                g = fs.tile([P, 4, NTF], F16, tag="g")
                for m4 in range(4):
                    bcol = dch * 4 + m4
                    u1ps = psU.tile([P, NTF], F32, tag="ups")
                    for kt in range(MT_RF):
                        nc.tensor.matmul(u1ps[:],
                                         vi1[:, kt, m4 * P:(m4 + 1) * P],
                                         w1T[:, kt, :],
                                         start=(kt == 0), stop=(kt == MT_RF - 1))
                    nc.scalar.activation(g[:, m4, :], u1ps[:],
                                         AF.Gelu_apprx_tanh,
                                         bias=bi1[:, bcol:bcol + 1])
                    u2ps = psU.tile([P, NTF], F32, tag="ups")
                    for kt in range(MT_RF):
                        nc.tensor.matmul(u2ps[:],
                                         vi2[:, kt, m4 * P:(m4 + 1) * P],
                                         w1T[:, kt, :],
                                         start=(kt == 0), stop=(kt == MT_RF - 1))
                    nc.vector.scalar_tensor_tensor(g[:, m4, :], u2ps[:],
                                                   bi2[:, bcol:bcol + 1],
                                                   g[:, m4, :], OP.add, OP.mult)
                for mr in range(MT_RF):
                    for ktl in range(MT_RF):
                        nc.tensor.matmul(tps[:, mr, :],
                                         uoc[:, ktl, mr * P:(mr + 1) * P],
                                         g[:, ktl, :],
                                         start=(dch == 0 and ktl == 0),
                                         stop=(dch == NDCH - 1 and ktl == MT_RF - 1),
                                         skip_group_check=True)
            tT = fs.tile([P, MT_RF, NTF], F16, tag="tT")
            nc.scalar.activation(tT[:], tps[:], AF.Copy)
            for tb in range(NTF // P):
                tt = tch * (NTF // P) + tb
                o_t = fs.tile([P, D], F32, tag="o_t")
                for c0 in range(0, D, 512):
                    c1 = min(c0 + 512, D)
                    yps = psY.tile([P, 512], F32, tag="yps")
                    for kt in range(MT_RF):
                        nc.tensor.matmul(yps[:, 0:c1 - c0],
                                         tT[:, kt, tb * P:(tb + 1) * P],
                                         vo_t[:, kt, c0:c1],
                                         start=(kt == 0), stop=False)
                    nc.tensor.matmul(yps[:, 0:c1 - c0], ones1[0:1, :],
                                     bo_r[0:1, c0:c1], start=False, stop=True)
                    nc.vector.tensor_tensor(o_t[:, c0:c1], yps[:, 0:c1 - c0],
                                            x1[:, tt, c0:c1], OP.add)
                nc.sync.dma_start(t["out"][tt * P:(tt + 1) * P, :], o_t[:])
    poolR_cm.__exit__(None, None, None)


def _build_module():
    nc = bacc.Bacc("TRN2", target_bir_lowering=False, debug=False, num_devices=N_CORES)
    t = _declare_io(nc)
    with tile.TileContext(nc) as tc:
        _emit(nc, tc, t)
    nc.compile()
    return nc


def _prep_weights(inputs):
    def rot_last(a):
        return np.concatenate([-a[..., HD // 2:], a[..., :HD // 2]], axis=-1)

    f32 = lambda a: np.ascontiguousarray(np.asarray(a), dtype=np.float32)
    bf = lambda a: np.ascontiguousarray(np.asarray(a, dtype=np.float32)
                                        .astype(np.float16))
    w = {}
    for p, U, V, b in (("q", inputs["Uq"], inputs["Vq"], inputs["bq"]),
                       ("k", inputs["Uk"], inputs["Vk"], inputs["bk"])):
        U, V, b = f32(U), f32(V), f32(b)
        ucat = U.transpose(1, 0, 2).reshape(D, HRA)           # [D, HRA]
        w[f"ucat_{p}"] = bf(ucat.reshape(MT_D, P, HRA).transpose(1, 0, 2))
        for suf, VV in ((p, V), (p + "r", rot_last(V))):
            blk = np.zeros((MT_D, P, P), np.float32)
            for m in range(MT_D):
                for j in range(2):
                    h = 2 * m + j
                    ro = (h % 4) * RA
                    blk[m, ro:ro + RA, 64 * j:64 * j + HD] = VV[h]
            w[f"bdv_{suf}"] = bf(blk.transpose(1, 0, 2))      # [P, MT_D, P]
        w[f"bias_{p}"] = f32(b.reshape(MT_D, P).T)
        w[f"bias_{p}r"] = f32(rot_last(b.reshape(H, HD)).reshape(MT_D, P).T)
    ucv = f32(inputs["Uv"]).transpose(1, 0, 2).reshape(D, HRA)
    w["ucat_v"] = bf(ucv.reshape(MT_D, P, HRA).transpose(1, 0, 2))
    bdvv = np.zeros((HRA, D), np.float32)
    Vv = f32(inputs["Vv"])
    for h in range(H):
        bdvv[h * RA:(h + 1) * RA, h * HD:(h + 1) * HD] = Vv[h]
    w["bdvv"] = bf(bdvv.reshape(KT_A, P, D).transpose(1, 0, 2))
    w["bv"] = f32(inputs["bv"])
    wot = f32(inputs["Wo_w"]).T                                # [D, D] (in, out)
    w["wotP"] = bf(wot.reshape(NPAIR, P, D).transpose(1, 0, 2))
    w["wob_row"] = bf(f32(inputs["Wo_b"]).reshape(1, D))
    w["bo_row"] = bf(f32(inputs["bo"]).reshape(1, D))
    ui = f32(inputs["Ui"])                                     # [D, RF]
    w["ui"] = bf(ui.reshape(MT_D, P, RF).transpose(1, 0, 2))
    vi = f32(inputs["Vi"])                                     # [RF, 2*DFF]
    w["vi"] = bf(vi.reshape(MT_RF, P, 2 * DFF).transpose(1, 0, 2))
    uo = f32(inputs["Uo"])                                     # [DFF, RF]
    w["uo"] = bf(uo.reshape(NDCH, MT_RF, P, RF).transpose(2, 0, 1, 3))
    vo = f32(inputs["Vo"])                                     # [RF, D]
    w["vo"] = bf(vo.reshape(MT_RF, P, D).transpose(1, 0, 2))
    bi = f32(inputs["bi"])
    w["bi1t"] = f32(bi[:DFF].reshape(MT_DFF, P).T)
    w["bi2t"] = f32(bi[DFF:].reshape(MT_DFF, P).T)
    return w


def _make_inmaps(inputs):
    w = _prep_weights(inputs)
    x = np.asarray(inputs["x"], dtype=np.float32)
    cos = np.asarray(inputs["cos"], dtype=np.float32)
    sin = np.asarray(inputs["sin"], dtype=np.float32)
    in_maps = []
    for core in range(N_CORES):
        b, hf = core // 2, core % 2
        sel = np.r_[hf * SQ:(hf + 1) * SQ, (1 - hf) * SQ:(2 - hf) * SQ]
        m = dict(w)
        m["xfull"] = np.ascontiguousarray(
            x[b][sel].astype(np.float16))
        cp, sp = cos[sel].T, sin[sel].T
        m["cos2"] = np.ascontiguousarray(
            np.concatenate([cp, cp], 0).astype(np.float16))
        m["sin2"] = np.ascontiguousarray(
            np.concatenate([sp, sp], 0).astype(np.float16))
        in_maps.append(m)
    return in_maps


def _run(inputs, **kwargs):
    nc = _CACHE.get("nc")
    if nc is None:
        nc = _CACHE["nc"] = _build_module()
    in_maps = _make_inmaps(inputs)
    res = run_bass_kernel_spmd(nc, in_maps, list(range(N_CORES)), **kwargs)
    out = np.empty((B, S, D), np.float32)
    for core in range(N_CORES):
        b, hf = core // 2, core % 2
        out[b, hf * SQ:(hf + 1) * SQ] = res.results[core]["out"]
    return out, res


def kernel(**inputs):
    out, _ = _run(inputs)
    return out


# revision 55
# speedup vs baseline: 2.3165x; 1.3897x over previous
"""Trainium2 Bass kernel for nn_ExplicitSVDBlock (dense transformer block).

Sharding: 8 NeuronCores = 4 batches x 2 query-halves of 1024 tokens.
Each core receives its batch's full 2048 tokens (permuted so its own
query tokens come first), redundantly builds K/V for all keys, and
computes everything else for its 1024 query tokens.  Zero cross-core
communication; host gathers the 8 [1024, 768] shards.

Device program (v2): bf16 activations/weights with fp32 PSUM accumulation,
DMA-xbar transposes for all token<->feature layout flips, fused
attention -> Wo -> LN2 pipeline per 256-token query chunk with pair-packed
token-major Wo (bias folded into a ones-row matmul), softmax denominators
broadcast via a PE rank-1 matmul, FFN weights chunk-streamed on the Pool
SWDGE queue.
"""
import sys

if '/opt/trn_rl_repo' not in sys.path:
    sys.path.insert(0, '/opt/trn_rl_repo')

import numpy as np
import ml_dtypes
import concourse.bass as bass
import concourse.bacc as bacc
import concourse.mybir as mybir
import concourse.tile as tile
from concourse.bass_utils import run_bass_kernel_spmd

F32 = mybir.dt.float32
BF = mybir.dt.bfloat16
F16 = mybir.dt.float16
AF = mybir.ActivationFunctionType
OP = mybir.AluOpType

B, S, D, H, HD, RA = 4, 2048, 768, 12, 64, 32
RF, DFF = 512, 3072
P = 128
SK, SQ = S, S // 2          # keys per core / queries per core
HRA = H * RA                # 384
MT_D = D // P               # 6
KT_A = HRA // P             # 3
NKT = SK // P               # 16
NQT = SQ // P               # 8
QCH = 256                   # attention query chunk
NQC = SQ // QCH             # 4
NPAIR = H // 2              # 6
MT_RF = RF // P             # 4
MT_DFF = DFF // P           # 24
NDCH = DFF // 512           # 6
NT = 256                    # FFN token chunk (one attention query chunk)
TCH = 512                   # phase-1 token chunk
SKH = SK // 2
LN_EPS = 1e-6
N_CORES = 8

_CACHE = {}

# All weights are shipped as two flat blobs (one per dtype) to minimize the
# number of executable arguments — axon dispatch overhead scales with the
# input-buffer count (~25-30us per input per dispatch).
W16_SPECS = [
    ("ucat_k", (P, MT_D, HRA)), ("ucat_v", (P, MT_D, HRA)),
    ("ucat_q", (P, MT_D, HRA)),
    ("bdv_k", (P, MT_D, P)), ("bdv_kr", (P, MT_D, P)),
    ("bdv_q", (P, MT_D, P)), ("bdv_qr", (P, MT_D, P)),
    ("bdvv", (P, KT_A, D)),
    ("wotP", (P, NPAIR, D)), ("wob_row", (1, D)), ("bo_row", (1, D)),
    ("ui", (P, MT_D, RF)), ("vi", (P, MT_RF, 2 * DFF)),
    ("uo", (P, NDCH, MT_RF, RF)), ("vo", (P, MT_RF, D)),
]
W32_SPECS = [
    ("bias_k", (P, MT_D)), ("bias_kr", (P, MT_D)),
    ("bias_q", (P, MT_D)), ("bias_qr", (P, MT_D)),
    ("bv", (D,)), ("bi1t", (P, MT_DFF)), ("bi2t", (P, MT_DFF)),
]


def _woffsets(specs):
    offs, n = {}, 0
    for name, shape in specs:
        offs[name] = (n, shape)
        n += int(np.prod(shape))
    return offs, n


W16_OFF, W16_N = _woffsets(W16_SPECS)
W32_OFF, W32_N = _woffsets(W32_SPECS)


def _declare_io(nc):
    t = {}
    t["xfull"] = nc.dram_tensor("xfull", [SK, D], F16, kind="ExternalInput")
    t["cs2"] = nc.dram_tensor("cs2", [2 * P, SK], F16, kind="ExternalInput")
    t["wf16"] = nc.dram_tensor("wf16", [W16_N], F16, kind="ExternalInput")
    t["wf32"] = nc.dram_tensor("wf32", [W32_N], F32, kind="ExternalInput")
    t["out"] = nc.dram_tensor("out", [SQ, D], F32, kind="ExternalOutput")
    return t


def _wap(t, name, sub=None):
    """AP into the flat weight blob for entry `name` (row-major dims)."""
    blob = "wf32" if name in W32_OFF else "wf16"
    off, shape = (W32_OFF if name in W32_OFF else W16_OFF)[name]
    if sub is not None:
        off, shape = sub(off, shape)
    dims = []
    stride = 1
    rdims = []
    for s in reversed(shape):
        rdims.append([stride, s])
        stride *= s
    dims = list(reversed(rdims))
    return bass.AP(t[blob].ap().tensor, off, dims)


def _bcast_row_ap(t, name, n):
    off, _ = W32_OFF[name]
    return bass.AP(t["wf32"].ap().tensor, off, [[0, P], [1, n]])


def _vi_ap(t, c0):
    off, _ = W16_OFF["vi"]
    return bass.AP(t["wf16"].ap().tensor, off + c0,
                   [[MT_RF * 2 * DFF, P], [2 * DFF, MT_RF], [1, 512]])


def _uo_ap(t, dch):
    off, _ = W16_OFF["uo"]
    return bass.AP(t["wf16"].ap().tensor, off + dch * MT_RF * RF,
                   [[NDCH * MT_RF * RF, P], [RF, MT_RF], [1, RF]])


def _emit(nc, tc, t):
    rsc = float(1.0 / np.sqrt(HD))

    poolR_cm = tc.tile_pool(name="res", bufs=1)
    poolR = poolR_cm.__enter__()
    qTr = poolR.tile([P, MT_D, SQ], F16)
    kTr = poolR.tile([P, MT_D, SK], F16)
    vaug = poolR.tile([P, NKT, H * (HD + 1)], BF)
    vaug4 = vaug[:].rearrange("p n (h e) -> p n h e", h=H)
    oT = poolR.tile([P, NPAIR, SQ], F16)
    x1 = poolR.tile([P, NQT, D], F16)
    h2T = poolR.tile([P, MT_D, SQ], F16)
    wotP = poolR.tile([P, NPAIR, D], F16)
    wob_r = poolR.tile([1, D], F16)
    bo_r = poolR.tile([1, D], F16)
    ones1 = poolR.tile([1, P], F16)
    ones_b = poolR.tile([1, HD], BF)
    ui_t = poolR.tile([P, MT_D, RF], F16)
    vo_t = poolR.tile([P, MT_RF, D], F16)
    bi1 = poolR.tile([P, MT_DFF], F32)
    bi2 = poolR.tile([P, MT_DFF], F32)
    eps_t = poolR.tile([P, 1], F32)

    nc.vector.memset(ones1[:], 1.0)
    nc.vector.memset(ones_b[:], 1.0)
    nc.vector.memset(eps_t[:], LN_EPS)

    # ---- phase 1: LN1 + QKV build ----
    with tc.tile_pool(name="bw", bufs=1) as wpool, \
         tc.tile_pool(name="bh", bufs=3) as hpool, \
         tc.tile_pool(name="bxu", bufs=2) as xupool, \
         tc.tile_pool(name="brot", bufs=2) as rotpool, \
         tc.tile_pool(name="bx", bufs=5) as xpool, \
         tc.tile_pool(name="bst", bufs=3) as stpool, \
         tc.tile_pool(name="psA", bufs=2, space="PSUM") as psA, \
         tc.tile_pool(name="psB", bufs=2, space="PSUM") as psB, \
         tc.tile_pool(name="psV", bufs=2, space="PSUM") as psV:

        # phase-1 weights on the Pool SWDGE queue (no HWDGE contention, Act
        # and SP stay free for the LN chain); k first: first projection used
        ucat, bdv, bias = {}, {}, {}
        for p in ("k", "v", "q"):
            w = wpool.tile([P, MT_D, HRA], F16, tag=f"ucat_{p}")
            ucat[p] = w
        for p in ("k", "kr", "q", "qr"):
            w = wpool.tile([P, MT_D, P], F16, tag=f"bdv_{p}")
            bdv[p] = w
            bl = wpool.tile([P, MT_D], F32, tag=f"bias_{p}")
            bias[p] = bl
        bdvv = wpool.tile([P, KT_A, D], F16)
        bv_bc = wpool.tile([P, D], F32)
        for p in ("k", "kr"):
            nc.gpsimd.dma_start(bdv[p][:], _wap(t, f"bdv_{p}"))
            nc.gpsimd.dma_start(bias[p][:], _wap(t, f"bias_{p}"))
        nc.gpsimd.dma_start(ucat["k"][:], _wap(t, "ucat_k"))
        nc.gpsimd.dma_start(ucat["v"][:], _wap(t, "ucat_v"))
        nc.gpsimd.dma_start(bdvv[:], _wap(t, "bdvv"))
        nc.gpsimd.dma_start(ucat["q"][:], _wap(t, "ucat_q"))
        for p in ("q", "qr"):
            nc.gpsimd.dma_start(bdv[p][:], _wap(t, f"bdv_{p}"))
            nc.gpsimd.dma_start(bias[p][:], _wap(t, f"bias_{p}"))
        nc.gpsimd.dma_start(bv_bc[:], _bcast_row_ap(t, "bv", D))
        ones_h = wpool.tile([P, H], BF)
        nc.vector.memset(ones_h[:], 1.0)
        for _kt in range(NKT):
            nc.vector.tensor_copy(vaug4[:, _kt, :, HD], ones_h[:])

        # software-prefetched x tiles (sync queue)
        xtiles = {}

        def load_x(gidx):
            xb = xpool.tile([P, D], F16, tag="xb")
            nc.sync.dma_start(xb[:], t["xfull"][gidx * P:(gidx + 1) * P, :])
            xtiles[gidx] = xb

        for _g in range(4):
            load_x(_g)

        chunks = [(h, c) for h in range(2) for c in range(SKH // TCH)]
        hts = {}

        def emit_prefetch():
            # resident weights for the later phases: issued after chunk 0 so
            # their transfers stay off the startup critical path
            nc.gpsimd.dma_start(wotP[:], _wap(t, "wotP"))
            nc.gpsimd.dma_start(wob_r[:], _wap(t, "wob_row"))
            nc.gpsimd.dma_start(bo_r[:], _wap(t, "bo_row"))
            nc.gpsimd.dma_start(ui_t[:], _wap(t, "ui"))
            nc.gpsimd.dma_start(vo_t[:], _wap(t, "vo"))
            nc.gpsimd.dma_start(bi1[:], _wap(t, "bi1t"))
            nc.gpsimd.dma_start(bi2[:], _wap(t, "bi2t"))

        def ln_chunk(ci):
            half, tch = chunks[ci]
            gcoff = half * SKH + tch * TCH
            hT = hpool.tile([P, MT_D, TCH], F16, tag="hT")
            cosc = hpool.tile([P, TCH], F16, tag="cosc")
            sinc = hpool.tile([P, TCH], F16, tag="sinc")
            nc.sync.dma_start(cosc[:], t["cs2"][0:P, gcoff:gcoff + TCH])
            nc.sync.dma_start(sinc[:], t["cs2"][P:2 * P, gcoff:gcoff + TCH])
            hts[ci] = (hT, cosc, sinc)
            for tb in range(TCH // P):
                g0 = gcoff // P + tb
                x_t = xtiles.pop(g0)
                xg = x_t[:].rearrange("p (n s) -> p n s", s=256)
                stats = stpool.tile([P, D // 256, 6], F32, tag="stats")
                for g in range(D // 256):
                    nc.vector.bn_stats(stats[:, g, :], xg[:, g, :])
                mv = stpool.tile([P, 2], F32, tag="mv")
                nc.vector.bn_aggr(mv[:], stats[:])
                sig = stpool.tile([P, 1], F32, tag="sig")
                nc.scalar.activation(sig[:], mv[:, 1:2], AF.Sqrt, bias=eps_t[:])
                nc.vector.reciprocal(sig[:], sig[:])
                xhat = xpool.tile([P, D], F16, tag="xhat")
                nc.vector.tensor_scalar(xhat[:], x_t[:], mv[:, 0:1], sig[:],
                                        OP.subtract, OP.mult)
                nc.sync.dma_start_transpose(hT[:, :, tb * P:(tb + 1) * P],
                                            xhat[:])
                if g0 + 4 < NKT:
                    load_x(g0 + 4)

        def proj_chunk(ci):
            half, tch = chunks[ci]
            coff = tch * TCH
            gcoff = half * SKH + coff
            hT, cosc, sinc = hts.pop(ci)
            projs = ["k", "v"] + (["q"] if half == 0 else [])
            for p in projs:
                    xs = xupool.tile([P, KT_A, TCH], F16, tag="xu_sb")
                    for ma in range(KT_A):
                        xps = psA.tile([P, TCH], F32, tag="b1")
                        for kt in range(MT_D):
                            nc.tensor.matmul(xps[:], ucat[p][:, kt, ma * P:(ma + 1) * P],
                                             hT[:, kt, :],
                                             start=(kt == 0), stop=(kt == MT_D - 1))
                        nc.scalar.activation(xs[:, ma, :], xps[:], AF.Copy)
                    if p == "v":
                        for tb in range(TCH // P):
                            vps = psV.tile([P, D], F32, tag="v_ps")
                            for n0 in range(0, D, 512):
                                n1 = min(n0 + 512, D)
                                for ka in range(KT_A):
                                    nc.tensor.matmul(vps[:, n0:n1],
                                                     xs[:, ka, tb * P:(tb + 1) * P],
                                                     bdvv[:, ka, n0:n1],
                                                     start=(ka == 0),
                                                     stop=(ka == KT_A - 1))
                            ktg = gcoff // P + tb
                            nc.vector.tensor_tensor(
                                vaug4[:, ktg, :, 0:HD],
                                vps[:].rearrange("p (h e) -> p h e", h=H),
                                bv_bc[:].rearrange("p (h e) -> p h e", h=H),
                                OP.add)
                    else:
                        dst = qTr if p == "q" else kTr
                        dcols = slice(coff, coff + TCH) if p == "q" else \
                                slice(gcoff, gcoff + TCH)
                        rot = rotpool.tile([P, MT_D, TCH], F16, tag="rot")
                        for m in range(MT_D):
                            ps2 = psB.tile([P, TCH], F32, tag="st2")
                            nc.tensor.matmul(ps2[:], bdv[p][:, m, :], xs[:, m // 2, :],
                                             start=True, stop=True)
                            nc.scalar.activation(dst[:, m, dcols], ps2[:], AF.Identity,
                                                 bias=bias[p][:, m:m + 1])
                            ps3 = psB.tile([P, TCH], F32, tag="st2")
                            nc.tensor.matmul(ps3[:], bdv[p + "r"][:, m, :],
                                             xs[:, m // 2, :], start=True, stop=True)
                            nc.vector.scalar_tensor_tensor(
                                rot[:, m, :], ps3[:], bias[p + "r"][:, m:m + 1],
                                sinc[:], OP.add, OP.mult)
                        dsl = dst[:, :, dcols]
                        cb = cosc[:, None, :].to_broadcast([P, MT_D, TCH])
                        nc.vector.tensor_tensor(dsl, dsl, cb, OP.mult)
                        nc.vector.tensor_tensor(dsl, dsl, rot[:], OP.add)

        # pipelined emission: LN(c+1) queued ahead of projections(c) so the
        # in-order Act/DVE queues never block the next chunk's LN chain
        for ci in range(len(chunks)):
            ln_chunk(ci)
            if ci == 0:
                emit_prefetch()
            if ci >= 1:
                proj_chunk(ci - 1)
        proj_chunk(len(chunks) - 1)

    # ---- attention phase: scores/exp/AV + Wo + residual (pure-Exp on Act) ----
    mv8 = poolR.tile([P, NQT, 2], F32)
    with tc.tile_pool(name="aexp", bufs=2, side="right") as apool, \
         tc.tile_pool(name="anrm", bufs=3, side="right") as npool, \
         tc.tile_pool(name="psS", bufs=2, space="PSUM") as psS, \
         tc.tile_pool(name="psO", bufs=2, space="PSUM") as psO, \
         tc.tile_pool(name="pw", bufs=2, space="PSUM") as pw:

        def emit_scores(h, qc, expS):
            pair, hh = divmod(h, 2)
            rs = slice(hh * 64, hh * 64 + 64)
            qcols = slice(qc * QCH, (qc + 1) * QCH)
            for kb in range(NKT // 4):
                sps = psS.tile([P, 4, QCH], F32, tag="s_ps")
                for j in range(4):
                    kt = kb * 4 + j
                    nc.tensor.matmul(sps[:, j, :],
                                     kTr[rs, pair, kt * P:(kt + 1) * P],
                                     qTr[rs, pair, qcols],
                                     start=True, stop=True)
                nc.scalar.activation(expS[:, kb * 4:(kb + 1) * 4, :],
                                     sps[:], AF.Exp, scale=rsc)

        def emit_av(h, qc, expS):
            pair, hh = divmod(h, 2)
            qcols = slice(qc * QCH, (qc + 1) * QCH)
            po = psO.tile([P, 2, QCH], F32, tag="o_ps")
            for kt in range(NKT):
                nc.tensor.matmul(po[0:HD + 1, 0, :], vaug4[:, kt, h, :],
                                 expS[:, kt, :],
                                 start=(kt == 0), stop=(kt == NKT - 1))
            srow = npool.tile([1, QCH], BF, tag="srow")
            with nc.allow_low_precision(reason="softmax denom recip in bf16"):
                nc.vector.reciprocal(srow[:], po[HD:HD + 1, 0, :])
            nc.tensor.matmul(po[0:HD, 1, :], ones_b[0:1, :], srow[0:1, :],
                             start=True, stop=True)
            rbs = npool.tile([HD, QCH], BF, tag="rbs")
            nc.vector.tensor_copy(rbs[:], po[0:HD, 1, :])
            nc.vector.tensor_tensor(oT[hh * 64:hh * 64 + 64, pair, qcols],
                                    po[0:HD, 0, :], rbs[:], OP.mult)

        def emit_attention(qc):
            prev = None
            for h in range(H):
                expS = apool.tile([P, NKT, QCH], BF, tag="expS")
                emit_scores(h, qc, expS)
                if prev is not None:
                    emit_av(h - 1, qc, prev)
                prev = expS
            emit_av(H - 1, qc, prev)

        def emit_wo(tc_):
            # Wo + residual into x1 (no LN here: keeps this phase pure-Exp)
            for tb in range(QCH // P):
                tt = tc_ * (QCH // P) + tb
                tok = slice(tc_ * QCH + tb * P, tc_ * QCH + (tb + 1) * P)
                xb2 = npool.tile([P, D], F16, tag="xq")
                nc.gpsimd.dma_start(xb2[:], t["xfull"][tt * P:(tt + 1) * P, :])
                for c0 in range(0, D, 512):
                    c1 = min(c0 + 512, D)
                    aps = pw.tile([P, 512], F32, tag="w")
                    for pr in range(NPAIR):
                        nc.tensor.matmul(aps[:, 0:c1 - c0], oT[:, pr, tok],
                                         wotP[:, pr, c0:c1],
                                         start=(pr == 0), stop=False)
                    nc.tensor.matmul(aps[:, 0:c1 - c0], ones1[0:1, :],
                                     wob_r[0:1, c0:c1], start=False, stop=True)
                    nc.vector.tensor_tensor(x1[:, tt, c0:c1], aps[:, 0:c1 - c0],
                                            xb2[:, c0:c1], OP.add)
                # LN2 stats on DVE only (sqrt batched later in the FFN phase)
                xg = x1[:, tt, :].rearrange("p (n s) -> p n s", s=256)
                stats = npool.tile([P, D // 256, 6], F32, tag="st3")
                for g in range(D // 256):
                    nc.vector.bn_stats(stats[:, g, :], xg[:, g, :])
                nc.vector.bn_aggr(mv8[:, tt, :], stats[:])

        for qc in range(NQC):
            emit_attention(qc)
            emit_wo(qc)

    # ---- FFN phase: LN2 prelude then low-rank GEGLU ----
    NTF = 512
    with tc.tile_pool(name="fn", bufs=3, side="right") as npool, \
         tc.tile_pool(name="fs", bufs=2, side="right") as fs, \
         tc.tile_pool(name="fcvi", bufs=2) as fcv, \
         tc.tile_pool(name="psU", bufs=2, space="PSUM") as psU, \
         tc.tile_pool(name="psT", bufs=1, space="PSUM") as psT, \
         tc.tile_pool(name="psY", bufs=2, space="PSUM") as psY:
        # LN2 normalize: one batched Sqrt (single table load), then DVE + DMA
        sig8 = npool.tile([P, NQT], F32, tag="sig8")
        nc.scalar.activation(sig8[:], mv8[:, :, 1], AF.Sqrt, bias=eps_t[:])
        nc.vector.reciprocal(sig8[:], sig8[:])
        for tt in range(NQT):
            h2b = npool.tile([P, D], F16, tag="h2b")
            nc.vector.tensor_scalar(h2b[:], x1[:, tt, :], mv8[:, tt, 0:1],
                                    sig8[:, tt:tt + 1], OP.subtract, OP.mult)
            nc.sync.dma_start_transpose(h2T[:, :, tt * P:(tt + 1) * P], h2b[:])

        for tch in range(SQ // NTF):
            tcols = slice(tch * NTF, (tch + 1) * NTF)
            w1T = fs.tile([P, MT_RF, NTF], F16, tag="w1T")
            for mt in range(MT_RF):
                wps = psU.tile([P, NTF], F32, tag="ups")
                for kt in range(MT_D):
                    nc.tensor.matmul(wps[:], ui_t[:, kt, mt * P:(mt + 1) * P],
                                     h2T[:, kt, tcols],
                                     start=(kt == 0), stop=(kt == MT_D - 1))
                nc.scalar.activation(w1T[:, mt, :], wps[:], AF.Copy)
            tps = psT.tile([P, MT_RF, NTF], F32, tag="t_ps")
            for dch in range(NDCH):
                vi1 = fcv.tile([P, MT_RF, 512], F16, tag="vi1")
                nc.gpsimd.dma_start(vi1[:], _vi_ap(t, dch * 512))
                vi2 = fcv.tile([P, MT_RF, 512], F16, tag="vi2")
                nc.gpsimd.dma_start(vi2[:], _vi_ap(t, DFF + dch * 512))
                uoc = fcv.tile([P, MT_RF, RF], F16, tag="uoc")
                nc.gpsimd.dma_start(uoc[:], _uo_ap(t, dch))
                g = fs.tile([P, 4, NTF], F16, tag="g")
                for m4 in range(4):
                    bcol = dch * 4 + m4
                    u1ps = psU.tile([P, NTF], F32, tag="ups")
                    for kt in range(MT_RF):
                        nc.tensor.matmul(u1ps[:],
                                         vi1[:, kt, m4 * P:(m4 + 1) * P],
                                         w1T[:, kt, :],
                                         start=(kt == 0), stop=(kt == MT_RF - 1))
                    nc.scalar.activation(g[:, m4, :], u1ps[:],
                                         AF.Gelu_apprx_tanh,
                                         bias=bi1[:, bcol:bcol + 1])
                    u2ps = psU.tile([P, NTF], F32, tag="ups")
                    for kt in range(MT_RF):
                        nc.tensor.matmul(u2ps[:],
                                         vi2[:, kt, m4 * P:(m4 + 1) * P],
                                         w1T[:, kt, :],
                                         start=(kt == 0), stop=(kt == MT_RF - 1))
                    nc.vector.scalar_tensor_tensor(g[:, m4, :], u2ps[:],
                                                   bi2[:, bcol:bcol + 1],
                                                   g[:, m4, :], OP.add, OP.mult)
                for mr in range(MT_RF):
                    for ktl in range(MT_RF):
                        nc.tensor.matmul(tps[:, mr, :],
                                         uoc[:, ktl, mr * P:(mr + 1) * P],
                                         g[:, ktl, :],
                                         start=(dch == 0 and ktl == 0),
                                         stop=(dch == NDCH - 1 and ktl == MT_RF - 1),
                                         skip_group_check=True)
            tT = fs.tile([P, MT_RF, NTF], F16, tag="tT")
            nc.scalar.activation(tT[:], tps[:], AF.Copy)
            for tb in range(NTF // P):
                tt = tch * (NTF // P) + tb
                o_t = fs.tile([P, D], F32, tag="o_t")
                for c0 in range(0, D, 512):
                    c1 = min(c0 + 512, D)
                    yps = psY.tile([P, 512], F32, tag="yps")
                    for kt in range(MT_RF):
                        nc.tensor.matmul(yps[:, 0:c1 - c0],
                                         tT[:, kt, tb * P:(tb + 1) * P],
                                         vo_t[:, kt, c0:c1],
                                         start=(kt == 0), stop=False)
                    nc.tensor.matmul(yps[:, 0:c1 - c0], ones1[0:1, :],
                                     bo_r[0:1, c0:c1], start=False, stop=True)
                    nc.vector.tensor_tensor(o_t[:, c0:c1], yps[:, 0:c1 - c0],
                                            x1[:, tt, c0:c1], OP.add)
                nc.sync.dma_start(t["out"][tt * P:(tt + 1) * P, :], o_t[:])
    poolR_cm.__exit__(None, None, None)


def _build_module():
    nc = bacc.Bacc("TRN2", target_bir_lowering=False, debug=False, num_devices=N_CORES)
    t = _declare_io(nc)
    with tile.TileContext(nc) as tc:
        _emit(nc, tc, t)
    nc.compile()
    return nc


def _prep_weights(inputs):
    def rot_last(a):
        return np.concatenate([-a[..., HD // 2:], a[..., :HD // 2]], axis=-1)

    f32 = lambda a: np.ascontiguousarray(np.asarray(a), dtype=np.float32)
    bf = lambda a: np.ascontiguousarray(np.asarray(a, dtype=np.float32)
                                        .astype(np.float16))
    w = {}
    for p, U, V, b in (("q", inputs["Uq"], inputs["Vq"], inputs["bq"]),
                       ("k", inputs["Uk"], inputs["Vk"], inputs["bk"])):
        U, V, b = f32(U), f32(V), f32(b)
        ucat = U.transpose(1, 0, 2).reshape(D, HRA)           # [D, HRA]
        w[f"ucat_{p}"] = bf(ucat.reshape(MT_D, P, HRA).transpose(1, 0, 2))
        for suf, VV in ((p, V), (p + "r", rot_last(V))):
            blk = np.zeros((MT_D, P, P), np.float32)
            for m in range(MT_D):
                for j in range(2):
                    h = 2 * m + j
                    ro = (h % 4) * RA
                    blk[m, ro:ro + RA, 64 * j:64 * j + HD] = VV[h]
            w[f"bdv_{suf}"] = bf(blk.transpose(1, 0, 2))      # [P, MT_D, P]
        w[f"bias_{p}"] = f32(b.reshape(MT_D, P).T)
        w[f"bias_{p}r"] = f32(rot_last(b.reshape(H, HD)).reshape(MT_D, P).T)
    ucv = f32(inputs["Uv"]).transpose(1, 0, 2).reshape(D, HRA)
    w["ucat_v"] = bf(ucv.reshape(MT_D, P, HRA).transpose(1, 0, 2))
    bdvv = np.zeros((HRA, D), np.float32)
    Vv = f32(inputs["Vv"])
    for h in range(H):
        bdvv[h * RA:(h + 1) * RA, h * HD:(h + 1) * HD] = Vv[h]
    w["bdvv"] = bf(bdvv.reshape(KT_A, P, D).transpose(1, 0, 2))
    w["bv"] = f32(inputs["bv"])
    wot = f32(inputs["Wo_w"]).T                                # [D, D] (in, out)
    w["wotP"] = bf(wot.reshape(NPAIR, P, D).transpose(1, 0, 2))
    w["wob_row"] = bf(f32(inputs["Wo_b"]).reshape(1, D))
    w["bo_row"] = bf(f32(inputs["bo"]).reshape(1, D))
    ui = f32(inputs["Ui"])                                     # [D, RF]
    w["ui"] = bf(ui.reshape(MT_D, P, RF).transpose(1, 0, 2))
    vi = f32(inputs["Vi"])                                     # [RF, 2*DFF]
    w["vi"] = bf(vi.reshape(MT_RF, P, 2 * DFF).transpose(1, 0, 2))
    uo = f32(inputs["Uo"])                                     # [DFF, RF]
    w["uo"] = bf(uo.reshape(NDCH, MT_RF, P, RF).transpose(2, 0, 1, 3))
    vo = f32(inputs["Vo"])                                     # [RF, D]
    w["vo"] = bf(vo.reshape(MT_RF, P, D).transpose(1, 0, 2))
    bi = f32(inputs["bi"])
    w["bi1t"] = f32(bi[:DFF].reshape(MT_DFF, P).T)
    w["bi2t"] = f32(bi[DFF:].reshape(MT_DFF, P).T)
    return w


def _make_inmaps(inputs):
    w = _prep_weights(inputs)
    for name, shape in W16_SPECS + W32_SPECS:
        assert tuple(w[name].shape) == tuple(shape), (name, w[name].shape, shape)
    wf16 = np.concatenate([w[n].ravel() for n, _ in W16_SPECS])
    assert wf16.dtype == np.float16 and wf16.size == W16_N
    wf32 = np.concatenate([w[n].ravel() for n, _ in W32_SPECS])
    assert wf32.dtype == np.float32 and wf32.size == W32_N
    x = np.asarray(inputs["x"], dtype=np.float32)
    cos = np.asarray(inputs["cos"], dtype=np.float32)
    sin = np.asarray(inputs["sin"], dtype=np.float32)
    in_maps = []
    for core in range(N_CORES):
        b, hf = core // 2, core % 2
        sel = np.r_[hf * SQ:(hf + 1) * SQ, (1 - hf) * SQ:(2 - hf) * SQ]
        m = {"wf16": wf16, "wf32": wf32}
        m["xfull"] = np.ascontiguousarray(x[b][sel].astype(np.float16))
        cp, sp = cos[sel].T, sin[sel].T
        m["cs2"] = np.ascontiguousarray(
            np.concatenate([cp, cp, sp, sp], 0).astype(np.float16))
        in_maps.append(m)
    return in_maps


def _run(inputs, **kwargs):
    nc = _CACHE.get("nc")
    if nc is None:
        nc = _CACHE["nc"] = _build_module()
    in_maps = _make_inmaps(inputs)
    res = run_bass_kernel_spmd(nc, in_maps, list(range(N_CORES)), **kwargs)
    out = np.empty((B, S, D), np.float32)
    for core in range(N_CORES):
        b, hf = core // 2, core % 2
        out[b, hf * SQ:(hf + 1) * SQ] = res.results[core]["out"]
    return out, res


def kernel(**inputs):
    out, _ = _run(inputs)
    return out


# revision 56
# speedup vs baseline: 2.6668x; 1.1512x over previous
"""Trainium2 Bass kernel for nn_ExplicitSVDBlock (dense transformer block).

Sharding: 8 NeuronCores = 4 batches x 2 query-halves of 1024 tokens.
Each core receives its batch's full 2048 tokens (permuted so its own
query tokens come first), redundantly builds K/V for all keys, and
computes everything else for its 1024 query tokens.  Zero cross-core
communication; host gathers the 8 [1024, 768] shards.

Device program: fp16 activations/weights with fp32 PSUM accumulation (bf16
only where exp's dynamic range requires it: expS, the V tensor it multiplies,
and the reciprocal denominator row), DMA-xbar transposes for all
token<->feature layout flips, attention phase kept pure-Exp on the Act engine
(all LN sqrt work batched elsewhere to avoid activation-table thrash),
pair-packed token-major Wo with the bias folded into a ones-row matmul,
softmax denominators broadcast via a PE rank-1 matmul, FFN weights
chunk-streamed on the Pool SWDGE queue, and all weights shipped as two flat
blobs so the executable takes 5 buffers instead of 25 (axon dispatch pays
~25-30us per input buffer per call).
"""
import sys

if '/opt/trn_rl_repo' not in sys.path:
    sys.path.insert(0, '/opt/trn_rl_repo')

import numpy as np
import ml_dtypes
import concourse.bass as bass
import concourse.bacc as bacc
import concourse.mybir as mybir
import concourse.tile as tile
from concourse.bass_utils import run_bass_kernel_spmd

F32 = mybir.dt.float32
BF = mybir.dt.bfloat16
F16 = mybir.dt.float16
AF = mybir.ActivationFunctionType
OP = mybir.AluOpType

B, S, D, H, HD, RA = 4, 2048, 768, 12, 64, 32
RF, DFF = 512, 3072
P = 128
SK, SQ = S, S // 2          # keys per core / queries per core
HRA = H * RA                # 384
MT_D = D // P               # 6
KT_A = HRA // P             # 3
NKT = SK // P               # 16
NQT = SQ // P               # 8
QCH = 256                   # attention query chunk
NQC = SQ // QCH             # 4
NPAIR = H // 2              # 6
MT_RF = RF // P             # 4
MT_DFF = DFF // P           # 24
NDCH = DFF // 512           # 6
NT = 256                    # FFN token chunk (one attention query chunk)
TCH = 512                   # phase-1 token chunk
SKH = SK // 2
LN_EPS = 1e-6
N_CORES = 8

_CACHE = {}

# All weights are shipped as two flat blobs (one per dtype) to minimize the
# number of executable arguments — axon dispatch overhead scales with the
# input-buffer count (~25-30us per input per dispatch).
W16_SPECS = [
    ("ucat_k", (P, MT_D, HRA)), ("ucat_v", (P, MT_D, HRA)),
    ("ucat_q", (P, MT_D, HRA)),
    ("bdv_k", (P, MT_D, P)), ("bdv_kr", (P, MT_D, P)),
    ("bdv_q", (P, MT_D, P)), ("bdv_qr", (P, MT_D, P)),
    ("bdvv", (P, KT_A, D)),
    ("wotP", (P, NPAIR, D)), ("wob_row", (1, D)), ("bo_row", (1, D)),
    ("ui", (P, MT_D, RF)), ("vi", (P, MT_RF, 2 * DFF)),
    ("uo", (P, NDCH, MT_RF, RF)), ("vo", (P, MT_RF, D)),
]
W32_SPECS = [
    ("bias_k", (P, MT_D)), ("bias_kr", (P, MT_D)),
    ("bias_q", (P, MT_D)), ("bias_qr", (P, MT_D)),
    ("bv", (D,)), ("bi1t", (P, MT_DFF)), ("bi2t", (P, MT_DFF)),
]


def _woffsets(specs):
    offs, n = {}, 0
    for name, shape in specs:
        offs[name] = (n, shape)
        n += int(np.prod(shape))
    return offs, n


W16_OFF, W16_N = _woffsets(W16_SPECS)
W32_OFF, W32_N = _woffsets(W32_SPECS)


def _declare_io(nc):
    t = {}
    t["xfull"] = nc.dram_tensor("xfull", [SK, D], F16, kind="ExternalInput")
    t["cs2"] = nc.dram_tensor("cs2", [2 * P, SK], F16, kind="ExternalInput")
    t["wf16"] = nc.dram_tensor("wf16", [W16_N], F16, kind="ExternalInput")
    t["wf32"] = nc.dram_tensor("wf32", [W32_N], F32, kind="ExternalInput")
    t["out"] = nc.dram_tensor("out", [SQ, D], F32, kind="ExternalOutput")
    return t


def _wap(t, name, sub=None):
    """AP into the flat weight blob for entry `name` (row-major dims)."""
    blob = "wf32" if name in W32_OFF else "wf16"
    off, shape = (W32_OFF if name in W32_OFF else W16_OFF)[name]
    if sub is not None:
        off, shape = sub(off, shape)
    dims = []
    stride = 1
    rdims = []
    for s in reversed(shape):
        rdims.append([stride, s])
        stride *= s
    dims = list(reversed(rdims))
    return bass.AP(t[blob].ap().tensor, off, dims)


def _bcast_row_ap(t, name, n):
    off, _ = W32_OFF[name]
    return bass.AP(t["wf32"].ap().tensor, off, [[0, P], [1, n]])


def _vi_ap(t, c0):
    off, _ = W16_OFF["vi"]
    return bass.AP(t["wf16"].ap().tensor, off + c0,
                   [[MT_RF * 2 * DFF, P], [2 * DFF, MT_RF], [1, 512]])


def _uo_ap(t, dch):
    off, _ = W16_OFF["uo"]
    return bass.AP(t["wf16"].ap().tensor, off + dch * MT_RF * RF,
                   [[NDCH * MT_RF * RF, P], [RF, MT_RF], [1, RF]])


def _emit(nc, tc, t):
    rsc = float(1.0 / np.sqrt(HD))

    poolR_cm = tc.tile_pool(name="res", bufs=1)
    poolR = poolR_cm.__enter__()
    qTr = poolR.tile([P, MT_D, SQ], F16)
    kTr = poolR.tile([P, MT_D, SK], F16)
    vaug = poolR.tile([P, NKT, H * (HD + 1)], BF)
    vaug4 = vaug[:].rearrange("p n (h e) -> p n h e", h=H)
    oT = poolR.tile([P, NPAIR, SQ], F16)
    x1 = poolR.tile([P, NQT, D], F16)
    h2T = poolR.tile([P, MT_D, SQ], F16)
    wotP = poolR.tile([P, NPAIR, D], F16)
    wob_r = poolR.tile([1, D], F16)
    bo_r = poolR.tile([1, D], F16)
    ones1 = poolR.tile([1, P], F16)
    ones_b = poolR.tile([1, HD], BF)
    ui_t = poolR.tile([P, MT_D, RF], F16)
    vo_t = poolR.tile([P, MT_RF, D], F16)
    bi1 = poolR.tile([P, MT_DFF], F32)
    bi2 = poolR.tile([P, MT_DFF], F32)
    eps_t = poolR.tile([P, 1], F32)

    nc.vector.memset(ones1[:], 1.0)
    nc.vector.memset(ones_b[:], 1.0)
    nc.vector.memset(eps_t[:], LN_EPS)

    # ---- phase 1: LN1 + QKV build ----
    with tc.tile_pool(name="bw", bufs=1) as wpool, \
         tc.tile_pool(name="bh", bufs=3) as hpool, \
         tc.tile_pool(name="bxu", bufs=2) as xupool, \
         tc.tile_pool(name="brot", bufs=2) as rotpool, \
         tc.tile_pool(name="bx", bufs=5) as xpool, \
         tc.tile_pool(name="bst", bufs=3) as stpool, \
         tc.tile_pool(name="psA", bufs=2, space="PSUM") as psA, \
         tc.tile_pool(name="psB", bufs=2, space="PSUM") as psB, \
         tc.tile_pool(name="psV", bufs=2, space="PSUM") as psV:

        # phase-1 weights on the Pool SWDGE queue (no HWDGE contention, Act
        # and SP stay free for the LN chain); k first: first projection used
        ucat, bdv, bias = {}, {}, {}
        for p in ("k", "v", "q"):
            w = wpool.tile([P, MT_D, HRA], F16, tag=f"ucat_{p}")
            ucat[p] = w
        for p in ("k", "kr", "q", "qr"):
            w = wpool.tile([P, MT_D, P], F16, tag=f"bdv_{p}")
            bdv[p] = w
            bl = wpool.tile([P, MT_D], F32, tag=f"bias_{p}")
            bias[p] = bl
        bdvv = wpool.tile([P, KT_A, D], F16)
        bv_bc = wpool.tile([P, D], F32)
        for p in ("k", "kr"):
            nc.gpsimd.dma_start(bdv[p][:], _wap(t, f"bdv_{p}"))
            nc.gpsimd.dma_start(bias[p][:], _wap(t, f"bias_{p}"))
        nc.gpsimd.dma_start(ucat["k"][:], _wap(t, "ucat_k"))
        nc.gpsimd.dma_start(ucat["v"][:], _wap(t, "ucat_v"))
        nc.gpsimd.dma_start(bdvv[:], _wap(t, "bdvv"))
        nc.gpsimd.dma_start(ucat["q"][:], _wap(t, "ucat_q"))
        for p in ("q", "qr"):
            nc.gpsimd.dma_start(bdv[p][:], _wap(t, f"bdv_{p}"))
            nc.gpsimd.dma_start(bias[p][:], _wap(t, f"bias_{p}"))
        nc.gpsimd.dma_start(bv_bc[:], _bcast_row_ap(t, "bv", D))
        ones_h = wpool.tile([P, H], BF)
        nc.vector.memset(ones_h[:], 1.0)
        for _kt in range(NKT):
            nc.vector.tensor_copy(vaug4[:, _kt, :, HD], ones_h[:])

        # software-prefetched x tiles (sync queue)
        xtiles = {}

        def load_x(gidx):
            xb = xpool.tile([P, D], F16, tag="xb")
            nc.sync.dma_start(xb[:], t["xfull"][gidx * P:(gidx + 1) * P, :])
            xtiles[gidx] = xb

        for _g in range(4):
            load_x(_g)

        chunks = [(h, c) for h in range(2) for c in range(SKH // TCH)]
        hts = {}

        def emit_prefetch():
            # resident weights for the later phases: issued after chunk 0 so
            # their transfers stay off the startup critical path
            nc.gpsimd.dma_start(wotP[:], _wap(t, "wotP"))
            nc.gpsimd.dma_start(wob_r[:], _wap(t, "wob_row"))
            nc.gpsimd.dma_start(bo_r[:], _wap(t, "bo_row"))
            nc.gpsimd.dma_start(ui_t[:], _wap(t, "ui"))
            nc.gpsimd.dma_start(vo_t[:], _wap(t, "vo"))
            nc.gpsimd.dma_start(bi1[:], _wap(t, "bi1t"))
            nc.gpsimd.dma_start(bi2[:], _wap(t, "bi2t"))

        def ln_chunk(ci):
            half, tch = chunks[ci]
            gcoff = half * SKH + tch * TCH
            hT = hpool.tile([P, MT_D, TCH], F16, tag="hT")
            cosc = hpool.tile([P, TCH], F16, tag="cosc")
            sinc = hpool.tile([P, TCH], F16, tag="sinc")
            nc.sync.dma_start(cosc[:], t["cs2"][0:P, gcoff:gcoff + TCH])
            nc.sync.dma_start(sinc[:], t["cs2"][P:2 * P, gcoff:gcoff + TCH])
            hts[ci] = (hT, cosc, sinc)
            for tb in range(TCH // P):
                g0 = gcoff // P + tb
                x_t = xtiles.pop(g0)
                xg = x_t[:].rearrange("p (n s) -> p n s", s=256)
                stats = stpool.tile([P, D // 256, 6], F32, tag="stats")
                for g in range(D // 256):
                    nc.vector.bn_stats(stats[:, g, :], xg[:, g, :])
                mv = stpool.tile([P, 2], F32, tag="mv")
                nc.vector.bn_aggr(mv[:], stats[:])
                sig = stpool.tile([P, 1], F32, tag="sig")
                nc.scalar.activation(sig[:], mv[:, 1:2], AF.Sqrt, bias=eps_t[:])
                nc.vector.reciprocal(sig[:], sig[:])
                xhat = xpool.tile([P, D], F16, tag="xhat")
                nc.vector.tensor_scalar(xhat[:], x_t[:], mv[:, 0:1], sig[:],
                                        OP.subtract, OP.mult)
                nc.sync.dma_start_transpose(hT[:, :, tb * P:(tb + 1) * P],
                                            xhat[:])
                if g0 + 4 < NKT:
                    load_x(g0 + 4)

        def proj_chunk(ci):
            half, tch = chunks[ci]
            coff = tch * TCH
            gcoff = half * SKH + coff
            hT, cosc, sinc = hts.pop(ci)
            projs = ["k", "v"] + (["q"] if half == 0 else [])
            for p in projs:
                    xs = xupool.tile([P, KT_A, TCH], F16, tag="xu_sb")
                    for ma in range(KT_A):
                        xps = psA.tile([P, TCH], F32, tag="b1")
                        for kt in range(MT_D):
                            nc.tensor.matmul(xps[:], ucat[p][:, kt, ma * P:(ma + 1) * P],
                                             hT[:, kt, :],
                                             start=(kt == 0), stop=(kt == MT_D - 1))
                        nc.scalar.activation(xs[:, ma, :], xps[:], AF.Copy)
                    if p == "v":
                        for tb in range(TCH // P):
                            vps = psV.tile([P, D], F32, tag="v_ps")
                            for n0 in range(0, D, 512):
                                n1 = min(n0 + 512, D)
                                for ka in range(KT_A):
                                    nc.tensor.matmul(vps[:, n0:n1],
                                                     xs[:, ka, tb * P:(tb + 1) * P],
                                                     bdvv[:, ka, n0:n1],
                                                     start=(ka == 0),
                                                     stop=(ka == KT_A - 1))
                            ktg = gcoff // P + tb
                            nc.vector.tensor_tensor(
                                vaug4[:, ktg, :, 0:HD],
                                vps[:].rearrange("p (h e) -> p h e", h=H),
                                bv_bc[:].rearrange("p (h e) -> p h e", h=H),
                                OP.add)
                    else:
                        dst = qTr if p == "q" else kTr
                        dcols = slice(coff, coff + TCH) if p == "q" else \
                                slice(gcoff, gcoff + TCH)
                        rot = rotpool.tile([P, MT_D, TCH], F16, tag="rot")
                        for m in range(MT_D):
                            ps2 = psB.tile([P, TCH], F32, tag="st2")
                            nc.tensor.matmul(ps2[:], bdv[p][:, m, :], xs[:, m // 2, :],
                                             start=True, stop=True)
                            nc.scalar.activation(dst[:, m, dcols], ps2[:], AF.Identity,
                                                 bias=bias[p][:, m:m + 1])
                            ps3 = psB.tile([P, TCH], F32, tag="st2")
                            nc.tensor.matmul(ps3[:], bdv[p + "r"][:, m, :],
                                             xs[:, m // 2, :], start=True, stop=True)
                            nc.vector.scalar_tensor_tensor(
                                rot[:, m, :], ps3[:], bias[p + "r"][:, m:m + 1],
                                sinc[:], OP.add, OP.mult)
                        dsl = dst[:, :, dcols]
                        cb = cosc[:, None, :].to_broadcast([P, MT_D, TCH])
                        nc.vector.tensor_tensor(dsl, dsl, cb, OP.mult)
                        nc.vector.tensor_tensor(dsl, dsl, rot[:], OP.add)

        # pipelined emission: LN(c+1) queued ahead of projections(c) so the
        # in-order Act/DVE queues never block the next chunk's LN chain
        for ci in range(len(chunks)):
            ln_chunk(ci)
            if ci == 0:
                emit_prefetch()
            if ci >= 1:
                proj_chunk(ci - 1)
        proj_chunk(len(chunks) - 1)

    # ---- attention phase: scores/exp/AV + Wo + residual (pure-Exp on Act) ----
    mv8 = poolR.tile([P, NQT, 2], F32)
    with tc.tile_pool(name="aexp", bufs=2, side="right") as apool, \
         tc.tile_pool(name="anrm", bufs=3, side="right") as npool, \
         tc.tile_pool(name="psS", bufs=2, space="PSUM") as psS, \
         tc.tile_pool(name="psO", bufs=2, space="PSUM") as psO, \
         tc.tile_pool(name="pw", bufs=2, space="PSUM") as pw:

        def emit_scores(h, qc, expS):
            pair, hh = divmod(h, 2)
            rs = slice(hh * 64, hh * 64 + 64)
            qcols = slice(qc * QCH, (qc + 1) * QCH)
            for kb in range(NKT // 4):
                sps = psS.tile([P, 4, QCH], F32, tag="s_ps")
                for j in range(4):
                    kt = kb * 4 + j
                    nc.tensor.matmul(sps[:, j, :],
                                     kTr[rs, pair, kt * P:(kt + 1) * P],
                                     qTr[rs, pair, qcols],
                                     start=True, stop=True)
                nc.scalar.activation(expS[:, kb * 4:(kb + 1) * 4, :],
                                     sps[:], AF.Exp, scale=rsc)

        def emit_av(h, qc, expS):
            pair, hh = divmod(h, 2)
            qcols = slice(qc * QCH, (qc + 1) * QCH)
            po = psO.tile([P, 2, QCH], F32, tag="o_ps")
            for kt in range(NKT):
                nc.tensor.matmul(po[0:HD + 1, 0, :], vaug4[:, kt, h, :],
                                 expS[:, kt, :],
                                 start=(kt == 0), stop=(kt == NKT - 1))
            srow = npool.tile([1, QCH], BF, tag="srow")
            with nc.allow_low_precision(reason="softmax denom recip in bf16"):
                nc.vector.reciprocal(srow[:], po[HD:HD + 1, 0, :])
            nc.tensor.matmul(po[0:HD, 1, :], ones_b[0:1, :], srow[0:1, :],
                             start=True, stop=True)
            rbs = npool.tile([HD, QCH], BF, tag="rbs")
            nc.vector.tensor_copy(rbs[:], po[0:HD, 1, :])
            nc.vector.tensor_tensor(oT[hh * 64:hh * 64 + 64, pair, qcols],
                                    po[0:HD, 0, :], rbs[:], OP.mult)

        def emit_attention(qc):
            prev = None
            for h in range(H):
                expS = apool.tile([P, NKT, QCH], BF, tag="expS")
                emit_scores(h, qc, expS)
                if prev is not None:
                    emit_av(h - 1, qc, prev)
                prev = expS
            emit_av(H - 1, qc, prev)

        def emit_wo(tc_):
            # Wo + residual into x1 (no LN here: keeps this phase pure-Exp)
            for tb in range(QCH // P):
                tt = tc_ * (QCH // P) + tb
                tok = slice(tc_ * QCH + tb * P, tc_ * QCH + (tb + 1) * P)
                xb2 = npool.tile([P, D], F16, tag="xq")
                nc.gpsimd.dma_start(xb2[:], t["xfull"][tt * P:(tt + 1) * P, :])
                for c0 in range(0, D, 512):
                    c1 = min(c0 + 512, D)
                    aps = pw.tile([P, 512], F32, tag="w")
                    for pr in range(NPAIR):
                        nc.tensor.matmul(aps[:, 0:c1 - c0], oT[:, pr, tok],
                                         wotP[:, pr, c0:c1],
                                         start=(pr == 0), stop=False)
                    nc.tensor.matmul(aps[:, 0:c1 - c0], ones1[0:1, :],
                                     wob_r[0:1, c0:c1], start=False, stop=True)
                    nc.vector.tensor_tensor(x1[:, tt, c0:c1], aps[:, 0:c1 - c0],
                                            xb2[:, c0:c1], OP.add)
                # LN2 stats on DVE only (sqrt batched later in the FFN phase)
                xg = x1[:, tt, :].rearrange("p (n s) -> p n s", s=256)
                stats = npool.tile([P, D // 256, 6], F32, tag="st3")
                for g in range(D // 256):
                    nc.vector.bn_stats(stats[:, g, :], xg[:, g, :])
                nc.vector.bn_aggr(mv8[:, tt, :], stats[:])

        for qc in range(NQC):
            emit_attention(qc)
            emit_wo(qc)

    # ---- FFN phase: LN2 prelude then low-rank GEGLU ----
    NTF = 512
    with tc.tile_pool(name="fn", bufs=3, side="right") as npool, \
         tc.tile_pool(name="fs", bufs=2, side="right") as fs, \
         tc.tile_pool(name="fcvi", bufs=2) as fcv, \
         tc.tile_pool(name="psU", bufs=2, space="PSUM") as psU, \
         tc.tile_pool(name="psT", bufs=1, space="PSUM") as psT, \
         tc.tile_pool(name="psY", bufs=2, space="PSUM") as psY:
        # LN2 normalize: one batched Sqrt (single table load), then DVE + DMA
        sig8 = npool.tile([P, NQT], F32, tag="sig8")
        nc.scalar.activation(sig8[:], mv8[:, :, 1], AF.Sqrt, bias=eps_t[:])
        nc.vector.reciprocal(sig8[:], sig8[:])
        for tt in range(NQT):
            h2b = npool.tile([P, D], F16, tag="h2b")
            nc.vector.tensor_scalar(h2b[:], x1[:, tt, :], mv8[:, tt, 0:1],
                                    sig8[:, tt:tt + 1], OP.subtract, OP.mult)
            nc.sync.dma_start_transpose(h2T[:, :, tt * P:(tt + 1) * P], h2b[:])

        for tch in range(SQ // NTF):
            tcols = slice(tch * NTF, (tch + 1) * NTF)
            w1T = fs.tile([P, MT_RF, NTF], F16, tag="w1T")
            for mt in range(MT_RF):
                wps = psU.tile([P, NTF], F32, tag="ups")
                for kt in range(MT_D):
                    nc.tensor.matmul(wps[:], ui_t[:, kt, mt * P:(mt + 1) * P],
                                     h2T[:, kt, tcols],
                                     start=(kt == 0), stop=(kt == MT_D - 1))
                nc.scalar.activation(w1T[:, mt, :], wps[:], AF.Copy)
            tps = psT.tile([P, MT_RF, NTF], F32, tag="t_ps")
            for dch in range(NDCH):
                vi1 = fcv.tile([P, MT_RF, 512], F16, tag="vi1")
                nc.gpsimd.dma_start(vi1[:], _vi_ap(t, dch * 512))
                vi2 = fcv.tile([P, MT_RF, 512], F16, tag="vi2")
                nc.gpsimd.dma_start(vi2[:], _vi_ap(t, DFF + dch * 512))
                uoc = fcv.tile([P, MT_RF, RF], F16, tag="uoc")
                nc.gpsimd.dma_start(uoc[:], _uo_ap(t, dch))
                g = fs.tile([P, 4, NTF], F16, tag="g")
                for m4 in range(4):
                    bcol = dch * 4 + m4
                    u1ps = psU.tile([P, NTF], F32, tag="ups")
                    for kt in range(MT_RF):
                        nc.tensor.matmul(u1ps[:],
                                         vi1[:, kt, m4 * P:(m4 + 1) * P],
                                         w1T[:, kt, :],
                                         start=(kt == 0), stop=(kt == MT_RF - 1))
                    nc.scalar.activation(g[:, m4, :], u1ps[:],
                                         AF.Gelu_apprx_tanh,
                                         bias=bi1[:, bcol:bcol + 1])
                    u2ps = psU.tile([P, NTF], F32, tag="ups")
                    for kt in range(MT_RF):
                        nc.tensor.matmul(u2ps[:],
                                         vi2[:, kt, m4 * P:(m4 + 1) * P],
                                         w1T[:, kt, :],
                                         start=(kt == 0), stop=(kt == MT_RF - 1))
                    nc.vector.scalar_tensor_tensor(g[:, m4, :], u2ps[:],
                                                   bi2[:, bcol:bcol + 1],
                                                   g[:, m4, :], OP.add, OP.mult)
                for mr in range(MT_RF):
                    for ktl in range(MT_RF):
                        nc.tensor.matmul(tps[:, mr, :],
                                         uoc[:, ktl, mr * P:(mr + 1) * P],
                                         g[:, ktl, :],
                                         start=(dch == 0 and ktl == 0),
                                         stop=(dch == NDCH - 1 and ktl == MT_RF - 1),
                                         skip_group_check=True)
            tT = fs.tile([P, MT_RF, NTF], F16, tag="tT")
            nc.scalar.activation(tT[:], tps[:], AF.Copy)
            for tb in range(NTF // P):
                tt = tch * (NTF // P) + tb
                o_t = fs.tile([P, D], F32, tag="o_t")
                for c0 in range(0, D, 512):
                    c1 = min(c0 + 512, D)
                    yps = psY.tile([P, 512], F32, tag="yps")
                    for kt in range(MT_RF):
                        nc.tensor.matmul(yps[:, 0:c1 - c0],
                                         tT[:, kt, tb * P:(tb + 1) * P],
                                         vo_t[:, kt, c0:c1],
                                         start=(kt == 0), stop=False)
                    nc.tensor.matmul(yps[:, 0:c1 - c0], ones1[0:1, :],
                                     bo_r[0:1, c0:c1], start=False, stop=True)
                    nc.vector.tensor_tensor(o_t[:, c0:c1], yps[:, 0:c1 - c0],
                                            x1[:, tt, c0:c1], OP.add)
                nc.sync.dma_start(t["out"][tt * P:(tt + 1) * P, :], o_t[:])
    poolR_cm.__exit__(None, None, None)


def _build_module():
    nc = bacc.Bacc("TRN2", target_bir_lowering=False, debug=False, num_devices=N_CORES)
    t = _declare_io(nc)
    with tile.TileContext(nc) as tc:
        _emit(nc, tc, t)
    nc.compile()
    return nc


def _prep_weights(inputs):
    def rot_last(a):
        return np.concatenate([-a[..., HD // 2:], a[..., :HD // 2]], axis=-1)

    f32 = lambda a: np.ascontiguousarray(np.asarray(a), dtype=np.float32)
    bf = lambda a: np.ascontiguousarray(np.asarray(a, dtype=np.float32)
                                        .astype(np.float16))
    w = {}
    for p, U, V, b in (("q", inputs["Uq"], inputs["Vq"], inputs["bq"]),
                       ("k", inputs["Uk"], inputs["Vk"], inputs["bk"])):
        U, V, b = f32(U), f32(V), f32(b)
        ucat = U.transpose(1, 0, 2).reshape(D, HRA)           # [D, HRA]
        w[f"ucat_{p}"] = bf(ucat.reshape(MT_D, P, HRA).transpose(1, 0, 2))
        for suf, VV in ((p, V), (p + "r", rot_last(V))):
            blk = np.zeros((MT_D, P, P), np.float32)
            for m in range(MT_D):
                for j in range(2):
                    h = 2 * m + j
                    ro = (h % 4) * RA
                    blk[m, ro:ro + RA, 64 * j:64 * j + HD] = VV[h]
            w[f"bdv_{suf}"] = bf(blk.transpose(1, 0, 2))      # [P, MT_D, P]
        w[f"bias_{p}"] = f32(b.reshape(MT_D, P).T)
        w[f"bias_{p}r"] = f32(rot_last(b.reshape(H, HD)).reshape(MT_D, P).T)
    ucv = f32(inputs["Uv"]).transpose(1, 0, 2).reshape(D, HRA)
    w["ucat_v"] = bf(ucv.reshape(MT_D, P, HRA).transpose(1, 0, 2))
    bdvv = np.zeros((HRA, D), np.float32)
    Vv = f32(inputs["Vv"])
    for h in range(H):
        bdvv[h * RA:(h + 1) * RA, h * HD:(h + 1) * HD] = Vv[h]
    w["bdvv"] = bf(bdvv.reshape(KT_A, P, D).transpose(1, 0, 2))
    w["bv"] = f32(inputs["bv"])
    wot = f32(inputs["Wo_w"]).T                                # [D, D] (in, out)
    w["wotP"] = bf(wot.reshape(NPAIR, P, D).transpose(1, 0, 2))
    w["wob_row"] = bf(f32(inputs["Wo_b"]).reshape(1, D))
    w["bo_row"] = bf(f32(inputs["bo"]).reshape(1, D))
    ui = f32(inputs["Ui"])                                     # [D, RF]
    w["ui"] = bf(ui.reshape(MT_D, P, RF).transpose(1, 0, 2))
    vi = f32(inputs["Vi"])                                     # [RF, 2*DFF]
    w["vi"] = bf(vi.reshape(MT_RF, P, 2 * DFF).transpose(1, 0, 2))
    uo = f32(inputs["Uo"])                                     # [DFF, RF]
    w["uo"] = bf(uo.reshape(NDCH, MT_RF, P, RF).transpose(2, 0, 1, 3))
    vo = f32(inputs["Vo"])                                     # [RF, D]
    w["vo"] = bf(vo.reshape(MT_RF, P, D).transpose(1, 0, 2))
    bi = f32(inputs["bi"])
    w["bi1t"] = f32(bi[:DFF].reshape(MT_DFF, P).T)
    w["bi2t"] = f32(bi[DFF:].reshape(MT_DFF, P).T)
    return w


def _make_inmaps(inputs):
    w = _prep_weights(inputs)
    for name, shape in W16_SPECS + W32_SPECS:
        assert tuple(w[name].shape) == tuple(shape), (name, w[name].shape, shape)
    wf16 = np.concatenate([w[n].ravel() for n, _ in W16_SPECS])
    assert wf16.dtype == np.float16 and wf16.size == W16_N
    wf32 = np.concatenate([w[n].ravel() for n, _ in W32_SPECS])
    assert wf32.dtype == np.float32 and wf32.size == W32_N
    x = np.asarray(inputs["x"], dtype=np.float32)
    cos = np.asarray(inputs["cos"], dtype=np.float32)
    sin = np.asarray(inputs["sin"], dtype=np.float32)
    in_maps = []
    for core in range(N_CORES):
        b, hf = core // 2, core % 2
        sel = np.r_[hf * SQ:(hf + 1) * SQ, (1 - hf) * SQ:(2 - hf) * SQ]
        m = {"wf16": wf16, "wf32": wf32}
        m["xfull"] = np.ascontiguousarray(x[b][sel].astype(np.float16))
        cp, sp = cos[sel].T, sin[sel].T
        m["cs2"] = np.ascontiguousarray(
            np.concatenate([cp, cp, sp, sp], 0).astype(np.float16))
        in_maps.append(m)
    return in_maps


def _run(inputs, **kwargs):
    nc = _CACHE.get("nc")
    if nc is None:
        nc = _CACHE["nc"] = _build_module()
    in_maps = _make_inmaps(inputs)
    res = run_bass_kernel_spmd(nc, in_maps, list(range(N_CORES)), **kwargs)
    out = np.empty((B, S, D), np.float32)
    for core in range(N_CORES):
        b, hf = core // 2, core % 2
        out[b, hf * SQ:(hf + 1) * SQ] = res.results[core]["out"]
    return out, res


def kernel(**inputs):
    out, _ = _run(inputs)
    return out
